# revision 1
# baseline (speedup 1.0000x reference)
"""GAT (3-layer, PyG GATConv-style) Trainium2 Bass kernel, 8-core SPMD.

Strategy (dst-sharded edge parallelism):
  - Pad N to NPAD (multiple of 1024). Core c owns node range [c*NPAD/8, (c+1)*NPAD/8),
    i.e. BPC = NPAD/1024 blocks of 128 dst nodes.
  - Host: append self-loops, sort edges by dst, assign each edge to the core that
    owns its dst, pad each (core, block) to T tiles of 128 edges. Indices/slots are
    shipped as per-core int16/bf16 tables; the device program is identical on all
    cores (same NEFF), only input data differs.
  - Per layer L: each core computes h_aug = x @ W_aug for its own nodes
    (W_aug has fused per-head attention projections a_src/a_dst as extra columns),
    stores rows [h | s_src | s_dst | pad] to DRAM, AllGather -> full table.
    Then per dst block: dma_gather rows by src (features+s_src) and by dst
    (s_dst slice); p = exp(leakyrelu(s_src+s_dst)); one-hot (edge->dst slot)
    matmuls aggregate numer = sum p*h and denom = sum p into PSUM; normalize,
    mean heads, +bias, relu -> next layer input (kept transposed in SBUF).
  - Layer 3 ends with a ones-vector matmul accumulating the node-mean partial;
    host sums the 8 per-core [1,128] partials.
"""

import numpy as np
import ml_dtypes

BF16 = ml_dtypes.bfloat16
NCORES = 8


# ----------------------------------------------------------------------------
# Host-side preprocessing
# ----------------------------------------------------------------------------

def _wrap16(idx_flat):
    """dma_gather index layout: [128, n/16] int16, idx i at [i%16, i//16],
    replicated across the 8 groups of 16 partitions."""
    n = idx_flat.shape[0]
    assert n % 16 == 0
    w = idx_flat.reshape(n // 16, 16).T.astype(np.int16)  # [16, n/16]
    return np.tile(w, (8, 1))  # [128, n/16]


def prep_static(edge_index, N, NPAD):
    """Edge structure -> per-core gather/slot tables. Returns (T, idxs, idxd, slot)."""
    E0 = edge_index.shape[1]
    loops = np.arange(N, dtype=np.int64)
    src = np.concatenate([edge_index[0].astype(np.int64), loops])
    dst = np.concatenate([edge_index[1].astype(np.int64), loops])
    order = np.argsort(dst, kind="stable")
    src_s, dst_s = src[order], dst[order]

    BPC = NPAD // (128 * NCORES)
    n_blocks = NPAD // 128
    NPC = NPAD // NCORES
    CR = 512 if NPC % 512 == 0 else NPC  # allgather chunk rows (<1MB/rank -> mesh)
    # hf row layout after chunked allgather: chunk k holds rank-c rows
    # [k*CR,(k+1)*CR) at hf rows k*CR*8 + c*CR + r%CR
    def node2row(n):
        c, r = n // NPC, n % NPC
        return (r // CR) * (CR * NCORES) + c * CR + (r % CR)
    # contiguous edge range per global block
    bounds = np.searchsorted(dst_s, np.arange(n_blocks + 1) * 128)
    counts = bounds[1:] - bounds[:-1]
    # per-block-index tile count: max over cores for that block position
    counts_cb = counts.reshape(NCORES, BPC)
    TBS = tuple(int(t) for t in np.maximum(
        1, np.ceil(counts_cb.max(axis=0) / 128).astype(np.int64)))

    idxs_cores, idxd_cores, slot_cores = [], [], []
    for c in range(NCORES):
        iw_cols, dw_cols, sl_cols = [], [], []
        for b in range(BPC):
            Tb = TBS[b]
            g = c * BPC + b
            lo, hi = int(bounds[g]), int(bounds[g + 1])
            n_e = hi - lo
            gsrc = np.zeros(Tb * 128, dtype=np.int64)
            gdst = np.zeros(Tb * 128, dtype=np.int64)
            gslot = np.full(Tb * 128, 255.0, dtype=np.float32)
            gsrc[:n_e] = node2row(src_s[lo:hi])
            gdst[:n_e] = node2row(dst_s[lo:hi])
            gslot[:n_e] = (dst_s[lo:hi] - g * 128).astype(np.float32)
            iw_cols.append(_wrap16(gsrc))
            dw_cols.append(_wrap16(gdst))
            # edge i of block -> (tile t=i//128, partition p=i%128)
            sl_cols.append(gslot.reshape(Tb, 128).T)  # [128, Tb]
        idxs_cores.append(np.concatenate(iw_cols, axis=1))
        idxd_cores.append(np.concatenate(dw_cols, axis=1))
        slot_cores.append(np.concatenate(sl_cols, axis=1).astype(np.float32))
    oh_cores = []
    for c in range(NCORES):
        sl = slot_cores[c]  # [128, sum(TBS)] float32
        oh = (sl[:, :, None] == np.arange(128, dtype=np.float32)[None, None, :])
        oh_cores.append(np.ascontiguousarray(
            oh.astype(BF16).reshape(128, -1)))  # [128, BPC*T*128]
    return TBS, idxs_cores, idxd_cores, oh_cores, CR


def prep_values(x, Ws, a_srcs, a_dsts, bs, NPAD):
    """Cast/fuse parameters. Returns dict of host arrays shared by all cores
    (except xT which is per-core sliced by the caller)."""
    N, F = x.shape
    xp = np.zeros((NPAD, F), dtype=np.float32)
    xp[:N] = x
    xT = np.ascontiguousarray(xp.T).astype(BF16)  # [F, NPAD]

    W_augs = []
    for W, a_s, a_d in zip(Ws, a_srcs, a_dsts):
        H, Fin, C = W.shape
        RW = _row_width(H, C)
        Wf = np.transpose(W, (1, 0, 2)).reshape(Fin, H * C)
        wsrc = np.einsum("hfc,hc->fh", W, a_s)
        wdst = np.einsum("hfc,hc->fh", W, a_d)
        off = H * C + (1 if H == 1 else 0)  # H==1: col H*C is the ones col
        Wa = np.zeros((Fin, RW), dtype=np.float32)
        Wa[:, : H * C] = Wf
        Wa[:, off : off + H] = wsrc
        Wa[:, off + H : off + 2 * H] = wdst
        W_augs.append(Wa.astype(BF16))
    return xT, W_augs


def _row_width(H, C):
    """h_aug row width (elements): H*C features + 2H scores, padded so the
    bf16 row is a multiple of 256 bytes (=128 elements)."""
    used = H * C + 2 * H
    return ((used + 127) // 128) * 128


# ----------------------------------------------------------------------------
# Device program
# ----------------------------------------------------------------------------

def build_nc(cfg, repeat=1):
    import concourse.bacc as bacc
    import concourse.bass as bass
    import concourse.mybir as mybir
    import concourse.tile as tile
    from concourse.masks import make_identity
    from contextlib import ExitStack

    f32 = mybir.dt.float32
    bf16 = mybir.dt.bfloat16
    i16 = mybir.dt.int16
    ALU = mybir.AluOpType
    ACT = mybir.ActivationFunctionType

    N = cfg["N"]
    NPAD = cfg["NPAD"]
    F_IN = cfg["F_IN"]
    C = cfg["C"]
    TBS = cfg["TBS"]            # tiles per block index
    SUMT = sum(TBS)
    tb_off = [0]
    for tb in TBS:
        tb_off.append(tb_off[-1] + tb)
    HS = cfg["HS"]              # heads per layer, e.g. (4, 4, 1)
    BPC = NPAD // (128 * NCORES)
    CR = cfg["CR"]
    NPC = NPAD // NCORES
    NCH = NPC // CR
    NL = len(HS)
    RWs = [_row_width(H, C) for H in HS]
    FINs = [F_IN] + [C] * (NL - 1)

    nc = bacc.Bacc("TRN2", target_bir_lowering=False, debug=False,
                   num_devices=NCORES)

    # ---- I/O ----
    xT_d = nc.dram_tensor("xT", [F_IN, NPAD // NCORES], bf16, kind="ExternalInput")
    idxs_d = nc.dram_tensor("idxs", [128, SUMT * 8], i16, kind="ExternalInput")
    idxd_d = nc.dram_tensor("idxd", [128, SUMT * 8], i16, kind="ExternalInput")
    oh_d = nc.dram_tensor("oh", [128, SUMT * 128], bf16, kind="ExternalInput")
    W_d = [nc.dram_tensor(f"w{i+1}", [FINs[i], RWs[i]], bf16, kind="ExternalInput")
           for i in range(NL)]
    bb_d = [nc.dram_tensor(f"bb{i+1}", [C, 1], f32, kind="ExternalInput")
            for i in range(NL - 1)]
    b3r_d = nc.dram_tensor("b3r", [1, C], f32, kind="ExternalInput")
    out_d = nc.dram_tensor("out", [1, C], f32, kind="ExternalOutput")

    with tile.TileContext(nc, num_cores=NCORES) as tc, ExitStack() as ctx:
        dram = ctx.enter_context(tc.tile_pool(name="dram", bufs=1, space="DRAM"))
        cpool = ctx.enter_context(tc.tile_pool(name="consts", bufs=1))
        gpool = ctx.enter_context(tc.tile_pool(name="gath", bufs=3))
        ohpool = ctx.enter_context(tc.tile_pool(name="oh", bufs=4))
        wpool = ctx.enter_context(tc.tile_pool(name="work", bufs=3))
        fpool = ctx.enter_context(tc.tile_pool(name="fin", bufs=2))
        hpool = ctx.enter_context(tc.tile_pool(name="haug", bufs=3))
        psum = ctx.enter_context(tc.tile_pool(name="ps", bufs=2, space="PSUM"))

        # DRAM scratch (pool tiles so Tile tracks collective/gather deps)
        hl = [dram.tile([NPAD // NCORES, RWs[i]], bf16, tag=f"hl{i}",
                        name=f"hl{i}") for i in range(NL)]
        hf = [dram.tile([NPAD, RWs[i]], bf16, tag=f"hf{i}", name=f"hf{i}")
              for i in range(NL)]

        # ---- constants into SBUF ----
        ident = cpool.tile([128, 128], f32, tag="ident")
        make_identity(nc, ident[:])
        xT_sb = cpool.tile([F_IN, NPAD // NCORES], bf16, tag="xT")
        nc.sync.dma_start(xT_sb[:], xT_d[:, :])
        idxs_sb = cpool.tile([128, SUMT * 8], i16, tag="idxs")
        nc.sync.dma_start(idxs_sb[:], idxs_d[:, :])
        idxd_sb = cpool.tile([128, SUMT * 8], i16, tag="idxd")
        nc.sync.dma_start(idxd_sb[:], idxd_d[:, :])
        W_sb = []
        for i in range(NL):
            w = cpool.tile([FINs[i], RWs[i]], bf16, tag=f"w{i}", name=f"w{i}")
            nc.sync.dma_start(w[:], W_d[i][:, :])
            W_sb.append(w)
        bb_sb = []
        for i in range(NL - 1):
            b = cpool.tile([C, 1], f32, tag=f"bb{i}", name=f"bb{i}")
            nc.sync.dma_start(b[:], bb_d[i][:, :])
            bb_sb.append(b)
        b3_sb = cpool.tile([1, C], f32, tag="b3")
        nc.sync.dma_start(b3_sb[:], b3r_d[:, :])
        ones_sb = cpool.tile([128, 1], f32, tag="ones")
        nc.vector.memset(ones_sb[:], 1.0)

        # next-layer transposed features, per layer boundary
        x2T = [cpool.tile([128, NPAD // NCORES], bf16, tag=f"x2T{i}",
                          name=f"x2T{i}") for i in range(NL - 1)]

        pfin = psum.tile([1, C], f32, tag="pfin", bufs=1)

        for _rep in range(repeat):
         for L in range(NL):
             H = HS[L]
             RW = RWs[L]
             SOFF = H * C + (1 if H == 1 else 0)  # s_src offset (H==1: skip ones col)
             S2 = 128                          # gather2 slice width (256B)
             s2off = (SOFF // 128) * 128       # aligned slice start covering s cols
             s_src_in2 = SOFF - s2off          # s_src position inside slice
             HC = H * C

             # ---- phase A: h_aug for own nodes ----
             for b in range(BPC):
                 if L == 0:
                     lhs = xT_sb[:, b * 128:(b + 1) * 128]
                 else:
                     lhs = x2T[L - 1][:, b * 128:(b + 1) * 128]
                 hs = hpool.tile([128, RW], bf16, tag="hs")
                 if RW > 512:
                     p1 = psum.tile([128, 512], f32, tag="pnum")
                     nc.tensor.matmul(p1[:], lhs, W_sb[L][:, 0:512],
                                      start=True, stop=True)
                     p2 = psum.tile([128, RW - 512], f32, tag="p128")
                     nc.tensor.matmul(p2[:], lhs, W_sb[L][:, 512:RW],
                                      start=True, stop=True)
                     nc.scalar.copy(hs[:, 0:512], p1[:])
                     nc.vector.tensor_copy(hs[:, 512:RW], p2[:])
                 else:
                     p1 = psum.tile([128, RW], f32, tag="pnum")
                     nc.tensor.matmul(p1[:], lhs, W_sb[L][:, 0:RW],
                                      start=True, stop=True)
                     nc.scalar.copy(hs[:, 0:RW], p1[:])
                 if H == 1:
                     nc.vector.memset(hs[:, HC:HC + 1], 1.0)
                 nc.sync.dma_start(hl[L][b * 128:(b + 1) * 128, :], hs[:])

             # ---- phase B: allgather, chunked <1MB/rank to stay on mesh algo ----
             for k in range(NCH):
                 nc.gpsimd.collective_compute(
                     "AllGather", mybir.AluOpType.bypass,
                     replica_groups=[list(range(NCORES))],
                     ins=[hl[L][k * CR:(k + 1) * CR, :].opt()],
                     outs=[hf[L][k * CR * NCORES:(k + 1) * CR * NCORES, :].opt()],
                 )

             # ---- phase C: edge aggregation per dst block ----
             GC = 6  # tiles per gather chunk (768 idxs = 48 desc/engine <= 64-desc packet limit)
             for b in range(BPC):
                 T = TBS[b]
                 base = tb_off[b]
                 chunks = [(c0, min(GC, T - c0)) for c0 in range(0, T, GC)]
                 NW = HC + 1 if H == 1 else HC  # H==1: denom rides as col C
                 numer = psum.tile([128, NW], f32, tag="pnum")
                 if H > 1:
                     denom = psum.tile([128, H], f32, tag="pden")
                 g1s, g2s = [], []
                 sc = wpool.tile([128, T, H], f32, tag="sc")
                 ohc = ohpool.tile([128, T * 128], bf16, tag="ohc", bufs=2)
                 nc.sync.dma_start(
                     ohc[:], oh_d[:, base * 128:(base + T) * 128])
                 for c0, tc_n in chunks:
                     ic = slice((base + c0) * 8, (base + c0 + tc_n) * 8)
                     g1 = gpool.tile([128, tc_n, RW], bf16, tag="g1", bufs=8)
                     nc.gpsimd.dma_gather(g1[:], hf[L][:, :], idxs_sb[:, ic],
                                          tc_n * 128, tc_n * 128, RW)
                     g2 = gpool.tile([128, tc_n, S2], bf16, tag="g2", bufs=8)
                     nc.gpsimd.dma_gather(g2[:], hf[L][:, s2off:s2off + S2],
                                          idxd_sb[:, ic], tc_n * 128, tc_n * 128,
                                          S2, elem_step=RW)
                     nc.vector.tensor_tensor(
                         sc[:, c0:c0 + tc_n, :], g1[:, :, SOFF:SOFF + H],
                         g2[:, :, s_src_in2 + H:s_src_in2 + 2 * H], ALU.add)
                     g1s.append(g1); g2s.append(g2)

                 # p = exp(leakyrelu(sc)), batched per block [128, T, H]
                 lr = wpool.tile([128, T, H], f32, tag="lr")
                 nc.vector.tensor_scalar(lr[:], sc[:], 0.2, None, op0=ALU.mult)
                 lr2 = wpool.tile([128, T, H], f32, tag="lr2")
                 nc.vector.tensor_tensor(lr2[:], lr[:], sc[:], ALU.max)
                 p = wpool.tile([128, T, H], f32, tag="p")
                 nc.scalar.activation(p[:], lr2[:], ACT.Exp)
                 if H > 1:
                     pb = wpool.tile([128, T, H], bf16, tag="pb")
                     nc.vector.tensor_copy(pb[:], p[:])

                 for ci, (c0, tc_n) in enumerate(chunks):
                     g1 = g1s[ci]
                     for tt in range(tc_n):
                         t = c0 + tt
                         oh_ap = ohc[:, t * 128:(t + 1) * 128]
                         msg = wpool.tile([128, NW], bf16, tag="msg")
                         if H == 1:
                             # one mul over [h | ones] -> [p*h | p]; one matmul
                             nc.vector.tensor_scalar(
                                 msg[:], g1[:, tt, 0:NW], p[:, t, 0:1],
                                 None, op0=ALU.mult)
                         else:
                             for h in range(H):
                                 src_ap = g1[:, tt, h * C:(h + 1) * C]
                                 dst_ap = msg[:, h * C:(h + 1) * C]
                                 pcol = p[:, t, h:h + 1]
                                 if h % 2 == 0:
                                     nc.vector.tensor_scalar(dst_ap, src_ap,
                                                             pcol, None,
                                                             op0=ALU.mult)
                                 else:
                                     nc.scalar.mul(dst_ap, src_ap, pcol)
                         nc.tensor.matmul(numer[:], oh_ap, msg[:],
                                          start=(t == 0), stop=(t == T - 1))
                         if H > 1:
                             nc.tensor.matmul(denom[:], oh_ap, pb[:, t, :],
                                              start=(t == 0), stop=(t == T - 1))

                 # ---- finalize block ----
                 dn = fpool.tile([128, H], f32, tag="dn")
                 dsrc = denom[:] if H > 1 else numer[:, HC:HC + 1]
                 nc.vector.tensor_scalar(dn[:], dsrc, float(H), 1e-16 * H,
                                         op0=ALU.mult, op1=ALU.add)
                 rc = fpool.tile([128, H], f32, tag="rc")
                 nc.vector.reciprocal(rc[:], dn[:])
                 if L < NL - 1:
                     ms = []
                     for h in range(H):
                         m = fpool.tile([128, C], f32, tag=f"m{h}", name=f"m{h}")
                         if h % 2 == 0:
                             nc.vector.tensor_scalar(
                                 m[:], numer[:, h * C:(h + 1) * C],
                                 rc[:, h:h + 1], None, op0=ALU.mult)
                         else:
                             nc.scalar.mul(m[:], numer[:, h * C:(h + 1) * C],
                                           rc[:, h:h + 1])
                         ms.append(m)
                     acc = ms[0]
                     if H > 1:
                         s01 = fpool.tile([128, C], f32, tag="s01")
                         nc.vector.tensor_tensor(s01[:], ms[0][:], ms[1][:], ALU.add)
                         acc = s01
                         if H == 4:
                             s23 = fpool.tile([128, C], f32, tag="s23")
                             nc.vector.tensor_tensor(s23[:], ms[2][:], ms[3][:],
                                                     ALU.add)
                             s4 = fpool.tile([128, C], f32, tag="s4")
                             nc.vector.tensor_tensor(s4[:], s01[:], s23[:], ALU.add)
                             acc = s4
                     pt = psum.tile([128, 128], f32, tag="p128")
                     nc.tensor.transpose(pt[:], acc[:], ident[:])
                     nc.scalar.activation(x2T[L][:, b * 128:(b + 1) * 128],
                                          pt[:], ACT.Relu, bias=bb_sb[L][:])
                 else:
                     o3 = fpool.tile([128, C], f32, tag="o3")
                     nc.vector.tensor_scalar(o3[:], numer[:, 0:C], rc[:, 0:1],
                                             None, op0=ALU.mult)
                     nc.tensor.matmul(pfin[:], ones_sb[:], o3[:],
                                      start=(b == 0), stop=(b == BPC - 1))

        fs = fpool.tile([1, C], f32, tag="fs")
        nc.vector.tensor_scalar(fs[:], pfin[:], 1.0 / N, None, op0=ALU.mult)
        fs2 = fpool.tile([1, C], f32, tag="fs2")
        nc.vector.tensor_tensor(fs2[:], fs[:], b3_sb[:], ALU.add)
        nc.sync.dma_start(out_d[:, :], fs2[:])

    nc.compile()
    return nc


# ----------------------------------------------------------------------------
# Entry points
# ----------------------------------------------------------------------------

def make_cfg_and_maps(inputs):
    x = np.asarray(inputs["x"])
    edge_index = np.asarray(inputs["edge_index"])
    N, F_IN = x.shape
    NPAD = ((N + 1023) // 1024) * 1024
    Ws = [np.asarray(inputs[f"W{i}"]) for i in (1, 2, 3)]
    a_srcs = [np.asarray(inputs[f"as{i}"]) for i in (1, 2, 3)]
    a_dsts = [np.asarray(inputs[f"ad{i}"]) for i in (1, 2, 3)]
    bs = [np.asarray(inputs[f"b{i}"]) for i in (1, 2, 3)]
    HS = tuple(W.shape[0] for W in Ws)
    C = Ws[0].shape[2]

    TBS, idxs_c, idxd_c, oh_c, CR = prep_static(edge_index, N, NPAD)
    xT, W_augs = prep_values(x, Ws, a_srcs, a_dsts, bs, NPAD)

    cfg = dict(N=N, NPAD=NPAD, F_IN=F_IN, C=C, TBS=TBS, HS=HS, CR=CR)
    NPC = NPAD // NCORES
    in_maps = []
    for c in range(NCORES):
        m = {
            "xT": np.ascontiguousarray(xT[:, c * NPC:(c + 1) * NPC]),
            "idxs": idxs_c[c],
            "idxd": idxd_c[c],
            "oh": oh_c[c],
            "b3r": (bs[2] * (1.0 / NCORES)).reshape(1, C).astype(np.float32),
        }
        for i in range(3):
            m[f"w{i+1}"] = W_augs[i]
        for i in range(2):
            m[f"bb{i+1}"] = bs[i].astype(np.float32).reshape(C, 1)
        in_maps.append(m)
    return cfg, in_maps


_NC_CACHE = {}


def _get_nc(cfg, repeat=1):
    key = (repeat,) + tuple(sorted((k, v if not isinstance(v, tuple) else v)
                                   for k, v in cfg.items()))
    if key not in _NC_CACHE:
        _NC_CACHE[key] = build_nc(cfg, repeat=repeat)
    return _NC_CACHE[key]


def run(inputs, trace=False, repeat=1, **kw):
    from concourse.bass_utils import run_bass_kernel_spmd
    cfg, in_maps = make_cfg_and_maps(inputs)
    nc = _get_nc(cfg, repeat=repeat)
    res = run_bass_kernel_spmd(nc, in_maps, core_ids=list(range(NCORES)),
                               trace=trace, **kw)
    out = np.zeros((1, cfg["C"]), dtype=np.float32)
    for r in res.results:
        out += r["out"]
    return out, res


def kernel(**inputs) -> np.ndarray:
    out, _ = run(inputs)
    return out



# revision 11
# speedup vs baseline: 15.8601x; 15.8601x over previous
"""GAT (3-layer, PyG GATConv-style) Trainium2 Bass kernel, 8-core SPMD.

Strategy (degree-bucketed dst-major fixed-degree layout):
  - Nodes are permuted by in-degree (desc) and assigned to (core, block,
    partition): chunk b of 1024 sorted nodes -> block b on every core.
    Per-block slot count D_b = max in-degree within the chunk (padded to a
    multiple of 8), so padding waste stays ~15%.
  - Per layer: each core computes h_aug = x @ W_aug for its 2560 nodes
    (W_aug fuses per-head a_src/a_dst projections as trailing columns),
    stores to DRAM, one AllGather -> full node table hf.
  - Phase C per block of 128 dst nodes: dma_gather pulls the D_b incident
    src rows per dst into [128 dst, D_b, RW] (slot-major index tables), then
    a handful of giant DVE ops do the whole block: p = exp(leakyrelu(
    s_src + s_dst)), numer = reduce_d(p * h), denom = reduce_d(p),
    out = head_mean(numer / denom).  Padding slots point at a poisoned row
    (s_src = -1e9 -> p = 0), so no masking is needed.
  - Layer boundary: out blocks stored node-major to DRAM; the next layer's
    transposed activations are re-loaded via dma_gather(transpose=True)
    with an identity index table.
  - Layer 3 ends with a ones-vector matmul accumulating the node-sum
    partial; host sums the 8 per-core [1,128] partials.
"""

import numpy as np
import ml_dtypes

BF16 = ml_dtypes.bfloat16
NCORES = 8
GC = 6  # slots per gather chunk (6*128 = 768 idxs = 48 desc/engine)


# ----------------------------------------------------------------------------
# Host-side preprocessing
# ----------------------------------------------------------------------------

def _wrap16(idx_flat):
    """dma_gather index layout: [128, n/16] int16, idx i at [i%16, i//16],
    replicated across the 8 groups of 16 partitions."""
    n = idx_flat.shape[0]
    assert n % 16 == 0
    w = idx_flat.reshape(n // 16, 16).T.astype(np.int16)  # [16, n/16]
    return np.tile(w, (8, 1))  # [128, n/16]


def _row_width(H, C):
    """h_aug row width: H*C features + 2H scores, padded so the bf16 row is
    a multiple of 256 bytes (dma_gather elem_size constraint)."""
    used = H * C + 2 * H
    return ((used + 127) // 128) * 128


def prep_static(edge_index, N, NPAD):
    """Degree-sorted node permutation + slot-major gather tables.

    Returns (Dpad, idx_cores, node_of_row, PAD_P0)."""
    loops = np.arange(N, dtype=np.int64)
    src = np.concatenate([edge_index[0].astype(np.int64), loops])
    dst = np.concatenate([edge_index[1].astype(np.int64), loops])
    deg = np.bincount(dst, minlength=NPAD)  # pad nodes have degree 0
    order = np.argsort(-deg, kind="stable")

    BPC = NPAD // (128 * NCORES)
    NPC = NPAD // NCORES
    node_of_row = np.empty(NPAD, dtype=np.int64)
    for b in range(BPC - 1):
        chunk = order[b * 1024:(b + 1) * 1024]
        q = np.arange(1024)
        rows = (q // 128) * NPC + b * 128 + (q % 128)
        node_of_row[rows] = chunk
    # last chunk: reals first on every core, pads fill the tail partitions
    last = order[(BPC - 1) * 1024:]
    n_real = int((deg[last] > 0).sum())
    assert n_real % NCORES == 0
    reals, pads = last[:n_real], last[n_real:]
    rpc = n_real // NCORES
    ppc = (1024 - n_real) // NCORES
    b = BPC - 1
    for c in range(NCORES):
        base = c * NPC + b * 128
        node_of_row[base:base + rpc] = reals[c * rpc:(c + 1) * rpc]
        node_of_row[base + rpc:base + 128] = pads[c * ppc:(c + 1) * ppc]
    row_of_node = np.empty(NPAD, dtype=np.int64)
    row_of_node[node_of_row] = np.arange(NPAD)
    PAD_P0 = rpc
    PADROW = NPAD  # dedicated poison row appended past the node table

    Dpad = []
    for b in range(BPC):
        mx = int(deg[order[b * 1024]])
        Dpad.append(max(GC, ((mx + GC - 1) // GC) * GC))

    sidx = np.argsort(dst, kind="stable")
    src_s, dst_s = src[sidx], dst[sidx]
    starts = np.searchsorted(dst_s, np.arange(NPAD))
    ends = np.searchsorted(dst_s, np.arange(NPAD) + 1)

    idx_cores = []
    for c in range(NCORES):
        cols = []
        for b in range(BPC):
            D = Dpad[b]
            flat = np.full(D * 128, PADROW, dtype=np.int64)
            for p in range(128):
                v = node_of_row[c * NPC + b * 128 + p]
                s0, s1 = starts[v], ends[v]
                k = s1 - s0
                if k:
                    flat[np.arange(k) * 128 + p] = row_of_node[src_s[s0:s1]]
            cols.append(_wrap16(flat))
        idx_cores.append(np.ascontiguousarray(np.concatenate(cols, axis=1)))
    return tuple(Dpad), idx_cores, node_of_row, PAD_P0


def prep_values(x, Ws, a_srcs, a_dsts, NPAD, node_of_row):
    N, F = x.shape
    xp = np.zeros((NPAD, F), dtype=np.float32)
    xp[:N] = x
    xperm = xp[node_of_row]  # row r holds node node_of_row[r]
    xT = np.ascontiguousarray(xperm.T).astype(BF16)  # [F, NPAD]

    W_augs = []
    for W, a_s, a_d in zip(Ws, a_srcs, a_dsts):
        H, Fin, C = W.shape
        RW = _row_width(H, C)
        Wf = np.transpose(W, (1, 0, 2)).reshape(Fin, H * C)
        wsrc = np.einsum("hfc,hc->fh", W, a_s)
        wdst = np.einsum("hfc,hc->fh", W, a_d)
        Wa = np.zeros((Fin, RW), dtype=np.float32)
        Wa[:, :H * C] = Wf
        Wa[:, H * C:H * C + H] = wsrc
        Wa[:, H * C + H:H * C + 2 * H] = wdst
        W_augs.append(Wa.astype(BF16))
    return xT, W_augs


# ----------------------------------------------------------------------------
# Device program
# ----------------------------------------------------------------------------

def build_nc(cfg, repeat=1):
    import concourse.bacc as bacc
    import concourse.mybir as mybir
    import concourse.tile as tile
    from contextlib import ExitStack

    f32 = mybir.dt.float32
    bf16 = mybir.dt.bfloat16
    i16 = mybir.dt.int16
    ALU = mybir.AluOpType
    ACT = mybir.ActivationFunctionType
    AX = mybir.AxisListType

    N = cfg["N"]
    NPAD = cfg["NPAD"]
    F_IN = cfg["F_IN"]
    C = cfg["C"]
    Dpad = cfg["Dpad"]
    HS = cfg["HS"]
    PAD_P0 = cfg["PAD_P0"]
    BPC = NPAD // (128 * NCORES)
    NPC = NPAD // NCORES
    NL = len(HS)
    RWs = [_row_width(H, C) for H in HS]
    FINs = [F_IN] + [C] * (NL - 1)
    DSUM = sum(Dpad)
    doff = [0]
    for d in Dpad:
        doff.append(doff[-1] + d)
    Dmax = max(Dpad)
    G1W = Dmax * max(RWs)

    nc = bacc.Bacc("TRN2", target_bir_lowering=False, debug=False,
                   num_devices=NCORES)

    xT_d = nc.dram_tensor("xT", [F_IN, NPC], bf16, kind="ExternalInput")
    idx_d = nc.dram_tensor("idxs", [128, DSUM * 8], i16, kind="ExternalInput")
    idt_d = nc.dram_tensor("idt", [128, NPC // 16], i16, kind="ExternalInput")
    W_d = [nc.dram_tensor(f"w{i+1}", [FINs[i], RWs[i]], bf16,
                          kind="ExternalInput") for i in range(NL)]
    bb_d = [nc.dram_tensor(f"bb{i+1}", [128, C], f32, kind="ExternalInput")
            for i in range(NL - 1)]
    b3_d = nc.dram_tensor("b3r", [1, C], f32, kind="ExternalInput")
    out_d = nc.dram_tensor("out", [1, C], f32, kind="ExternalOutput")

    with tile.TileContext(nc, num_cores=NCORES) as tc, ExitStack() as ctx:
        dram = ctx.enter_context(tc.tile_pool(name="dram", bufs=1, space="DRAM"))
        cpool = ctx.enter_context(tc.tile_pool(name="consts", bufs=1))
        hpool = ctx.enter_context(tc.tile_pool(name="hs", bufs=2))
        wpool = ctx.enter_context(tc.tile_pool(name="work", bufs=2))
        psum = ctx.enter_context(tc.tile_pool(name="ps", bufs=2, space="PSUM"))

        hl = [dram.tile([NPC, RWs[i]], bf16, tag=f"hl{i}", name=f"hl{i}")
              for i in range(NL)]
        # one extra row past the node table: the poison row pad slots point at
        hf = [dram.tile([NPAD + 1, RWs[i]], bf16, tag=f"hf{i}", name=f"hf{i}")
              for i in range(NL)]
        x2d = [dram.tile([NPC, C], bf16, tag=f"x2d{i}", name=f"x2d{i}")
               for i in range(NL - 1)]

        xT_sb = cpool.tile([F_IN, NPC], bf16, tag="xT")
        nc.sync.dma_start(xT_sb[:], xT_d[:, :])
        idx_sb = cpool.tile([128, DSUM * 8], i16, tag="idx")
        nc.sync.dma_start(idx_sb[:], idx_d[:, :])
        idt_sb = cpool.tile([128, NPC // 16], i16, tag="idt")
        nc.sync.dma_start(idt_sb[:], idt_d[:, :])
        W_sb = []
        for i in range(NL):
            w = cpool.tile([FINs[i], RWs[i]], bf16, tag=f"w{i}", name=f"w{i}")
            nc.sync.dma_start(w[:], W_d[i][:, :])
            W_sb.append(w)
        bb_sb = []
        for i in range(NL - 1):
            t = cpool.tile([128, C], f32, tag=f"bb{i}", name=f"bb{i}")
            nc.sync.dma_start(t[:], bb_d[i][:, :])
            bb_sb.append(t)
        b3_sb = cpool.tile([1, C], f32, tag="b3")
        nc.sync.dma_start(b3_sb[:], b3_d[:, :])
        ones_sb = cpool.tile([128, 1], f32, tag="ones")
        nc.vector.memset(ones_sb[:], 1.0)
        pois_sb = cpool.tile([1, max(RWs)], bf16, tag="pois")
        nc.vector.memset(pois_sb[:], -1e9)
        x2T = [cpool.tile([C, NPC], bf16, tag=f"x2T{i}", name=f"x2T{i}")
               for i in range(NL - 1)]
        g1f = cpool.tile([128, G1W], bf16, tag="g1f")
        msgf = cpool.tile([128, max(HS) * C * Dmax], bf16, tag="msgf")

        pfin = psum.tile([1, C], f32, tag="pfin", bufs=1)

        for _rep in range(repeat):
         for L in range(NL):
            H = HS[L]
            RW = RWs[L]
            SOFF = H * C

            # ---- phase A: h_aug for own nodes ----
            if L > 0:
                # transposed activations via identity-index transpose-gather
                for k0 in range(0, NPC, 768):
                    kn = min(768, NPC - k0)
                    nc.gpsimd.dma_gather(
                        x2T[L - 1][:, k0:k0 + kn].unsqueeze(1),
                        x2d[L - 1][:, :], idt_sb[:, k0 // 16:(k0 + kn) // 16],
                        kn, kn, C, transpose=True)
            lsrc = xT_sb if L == 0 else x2T[L - 1]
            hs4 = None
            for b in range(BPC):
                j = b % 4
                if j == 0:
                    hs4 = hpool.tile([128, 4, RW], bf16, tag="hs4")
                lhsT = lsrc[:, b * 128:(b + 1) * 128]
                if RW > 512:
                    pA = psum.tile([128, 512], f32, tag="pA")
                    nc.tensor.matmul(pA[:], lhsT, W_sb[L][:, 0:512],
                                     start=True, stop=True)
                    pB = psum.tile([128, RW - 512], f32, tag="pB")
                    nc.tensor.matmul(pB[:], lhsT, W_sb[L][:, 512:RW],
                                     start=True, stop=True)
                    nc.scalar.copy(hs4[:, j, 0:512], pA[:])
                    nc.vector.tensor_copy(hs4[:, j, 512:RW], pB[:])
                else:
                    pA = psum.tile([128, RW], f32, tag="pA")
                    nc.tensor.matmul(pA[:], lhsT, W_sb[L][:, 0:RW],
                                     start=True, stop=True)
                    nc.scalar.copy(hs4[:, j, :], pA[:])
                if j == 3:
                    dst_ap = hl[L][(b - 3) * 128:(b + 1) * 128, :].rearrange(
                        "(j p) w -> p j w", j=4)
                    nc.sync.dma_start(dst_ap, hs4[:])

            # ---- phase B: allgather + poison row ----
            nc.gpsimd.collective_compute(
                "AllGather", mybir.AluOpType.bypass,
                replica_groups=[list(range(NCORES))],
                ins=[hl[L][:, :].opt()],
                outs=[hf[L][0:NPAD, :].opt()],
            )
            nc.sync.dma_start(hf[L][NPAD:NPAD + 1, :], pois_sb[:, 0:RW])
            # own nodes' s_dst, [128, BPC, H]
            sdl = wpool.tile([128, BPC, H], bf16, tag="sdl")
            hl_ap = hl[L][:, :].rearrange("(b p) w -> p b w", p=128)
            nc.sync.dma_start(sdl[:], hl_ap[:, :, SOFF + H:SOFF + 2 * H])

            # ---- phase C: per dst block ----
            for b in range(BPC):
                D = Dpad[b]
                g1 = g1f[:].rearrange("p (d w) -> p d w", w=RW)[:, 0:D, :]
                for s0 in range(0, D, GC):
                    ns = min(GC, D - s0)
                    nc.gpsimd.dma_gather(
                        g1[:, s0:s0 + ns, :], hf[L][:, :],
                        idx_sb[:, (doff[b] + s0) * 8:(doff[b] + s0 + ns) * 8],
                        ns * 128, ns * 128, RW)
                sc = wpool.tile([128, D, H], f32, tag="sc")
                nc.vector.tensor_tensor(
                    sc[:], g1[:, :, SOFF:SOFF + H],
                    sdl[:, b:b + 1, :].broadcast_to([128, D, H]), ALU.add)
                lr = wpool.tile([128, D, H], f32, tag="lr")
                nc.vector.scalar_tensor_tensor(lr[:], sc[:], 0.2, sc[:],
                                               op0=ALU.mult, op1=ALU.max)
                p = wpool.tile([128, D, H], f32, tag="p")
                nc.scalar.activation(p[:], lr[:], ACT.Exp)

                msg = msgf[:, 0:H * C * D].rearrange(
                    "p (h c d) -> p h c d", h=H, c=C)
                g1v = g1[:, :, 0:SOFF].rearrange("p d (h c) -> p h c d", h=H)
                pv = p[:].rearrange("p d h -> p h d").unsqueeze(2).broadcast_to(
                    [128, H, C, D])
                nc.vector.tensor_tensor(msg, g1v, pv, ALU.mult)
                nm = wpool.tile([128, H, C], f32, tag="nm")
                nc.vector.tensor_reduce(nm[:], msg, axis=AX.X, op=ALU.add)
                dn0 = wpool.tile([128, H], f32, tag="dn0")
                nc.vector.tensor_reduce(dn0[:], p[:].rearrange("p d h -> p h d"),
                                        axis=AX.X, op=ALU.add)
                dn = wpool.tile([128, H], f32, tag="dn")
                nc.vector.tensor_scalar(dn[:], dn0[:], float(H), 1e-16 * H,
                                        op0=ALU.mult, op1=ALU.add)
                rc = wpool.tile([128, H], f32, tag="rc")
                nc.vector.reciprocal(rc[:], dn[:])
                if L < NL - 1:
                    nm2 = wpool.tile([128, H, C], f32, tag="nm2")
                    nc.vector.tensor_tensor(
                        nm2[:], nm[:],
                        rc[:].unsqueeze(2).broadcast_to([128, H, C]), ALU.mult)
                    xo = wpool.tile([128, C], f32, tag="xo")
                    nc.vector.tensor_reduce(xo[:],
                                            nm2[:].rearrange("p h c -> p c h"),
                                            axis=AX.X, op=ALU.add)
                    xb = wpool.tile([128, C], f32, tag="xb")
                    nc.vector.tensor_tensor(xb[:], xo[:], bb_sb[L][:, :],
                                            ALU.add)
                    x2b = wpool.tile([128, C], bf16, tag="x2b")
                    nc.vector.tensor_scalar(x2b[:], xb[:], 0.0, None,
                                            op0=ALU.max)
                    nc.sync.dma_start(x2d[L][b * 128:(b + 1) * 128, :], x2b[:])
                else:
                    o3 = wpool.tile([128, C], f32, tag="o3")
                    nc.vector.tensor_scalar(o3[:], nm[:, 0, :], rc[:, 0:1],
                                            None, op0=ALU.mult)
                    nc.tensor.matmul(pfin[:], ones_sb[:], o3[:],
                                     start=(b == 0), stop=(b == BPC - 1))

        fs = wpool.tile([1, C], f32, tag="fs")
        nc.vector.tensor_scalar(fs[:], pfin[:], 1.0 / N, None, op0=ALU.mult)
        fs2 = wpool.tile([1, C], f32, tag="fs2")
        nc.vector.tensor_tensor(fs2[:], fs[:], b3_sb[:], ALU.add)
        nc.sync.dma_start(out_d[:, :], fs2[:])

    nc.compile()
    return nc


# ----------------------------------------------------------------------------
# Entry points
# ----------------------------------------------------------------------------

_PREP_CACHE = {}


def make_cfg_and_maps(inputs):
    x = np.asarray(inputs["x"])
    edge_index = np.asarray(inputs["edge_index"])
    N, F_IN = x.shape
    NPAD = ((N + 1023) // 1024) * 1024
    Ws = [np.asarray(inputs[f"W{i}"]) for i in (1, 2, 3)]
    a_srcs = [np.asarray(inputs[f"as{i}"]) for i in (1, 2, 3)]
    a_dsts = [np.asarray(inputs[f"ad{i}"]) for i in (1, 2, 3)]
    bs = [np.asarray(inputs[f"b{i}"]) for i in (1, 2, 3)]
    HS = tuple(W.shape[0] for W in Ws)
    C = Ws[0].shape[2]

    pkey = (id(inputs["edge_index"]), edge_index.shape, N)
    if pkey not in _PREP_CACHE:
        _PREP_CACHE.clear()
        _PREP_CACHE[pkey] = prep_static(edge_index, N, NPAD)
    Dpad, idx_cores, node_of_row, PAD_P0 = _PREP_CACHE[pkey]
    xT, W_augs = prep_values(x, Ws, a_srcs, a_dsts, NPAD, node_of_row)

    cfg = dict(N=N, NPAD=NPAD, F_IN=F_IN, C=C, Dpad=Dpad, HS=HS,
               PAD_P0=PAD_P0)
    NPC = NPAD // NCORES
    idt = _wrap16(np.arange(NPC, dtype=np.int64))
    in_maps = []
    for c in range(NCORES):
        m = {
            "xT": np.ascontiguousarray(xT[:, c * NPC:(c + 1) * NPC]),
            "idxs": idx_cores[c],
            "idt": idt,
            "b3r": (bs[2] * (1.0 / NCORES)).reshape(1, C).astype(np.float32),
        }
        for i in range(3):
            m[f"w{i+1}"] = W_augs[i]
        for i in range(2):
            m[f"bb{i+1}"] = np.ascontiguousarray(
                np.broadcast_to(bs[i].astype(np.float32)[None, :], (128, C)))
        in_maps.append(m)
    return cfg, in_maps


_NC_CACHE = {}


def _get_nc(cfg, repeat=1):
    key = (repeat,) + tuple(sorted((k, v) for k, v in cfg.items()))
    if key not in _NC_CACHE:
        _NC_CACHE[key] = build_nc(cfg, repeat=repeat)
    return _NC_CACHE[key]


def run(inputs, trace=False, repeat=1, **kw):
    from concourse.bass_utils import run_bass_kernel_spmd
    cfg, in_maps = make_cfg_and_maps(inputs)
    nc = _get_nc(cfg, repeat=repeat)
    res = run_bass_kernel_spmd(nc, in_maps, core_ids=list(range(NCORES)),
                               trace=trace, **kw)
    out = np.zeros((1, cfg["C"]), dtype=np.float32)
    for r in res.results:
        out += r["out"]
    return out, res


def kernel(**inputs) -> np.ndarray:
    out, _ = run(inputs)
    return out


# revision 13
# speedup vs baseline: 19.0882x; 1.2035x over previous
"""GAT (3-layer, PyG GATConv-style) Trainium2 Bass kernel, 8-core SPMD.

Strategy (degree-bucketed dst-major fixed-degree layout):
  - Nodes are permuted by in-degree (desc) and assigned to (core, block,
    partition): chunk b of 1024 sorted nodes -> block b on every core.
    Per-block slot count D_b = max in-degree within the chunk (padded to a
    multiple of 8), so padding waste stays ~15%.
  - Per layer: each core computes h_aug = x @ W_aug for its 2560 nodes
    (W_aug fuses per-head a_src/a_dst projections as trailing columns),
    stores to DRAM, one AllGather -> full node table hf.
  - Phase C per block of 128 dst nodes: dma_gather pulls the D_b incident
    src rows per dst into [128 dst, D_b, RW] (slot-major index tables), then
    a handful of giant DVE ops do the whole block: p = exp(leakyrelu(
    s_src + s_dst)), numer = reduce_d(p * h), denom = reduce_d(p),
    out = head_mean(numer / denom).  Padding slots point at a poisoned row
    (s_src = -1e9 -> p = 0), so no masking is needed.
  - Layer boundary: out blocks stored node-major to DRAM; the next layer's
    transposed activations are re-loaded via dma_gather(transpose=True)
    with an identity index table.
  - Layer 3 ends with a ones-vector matmul accumulating the node-sum
    partial; host sums the 8 per-core [1,128] partials.
"""

import numpy as np
import ml_dtypes

BF16 = ml_dtypes.bfloat16
NCORES = 8
GC = 6  # slots per gather chunk (6*128 = 768 idxs = 48 desc/engine)


# ----------------------------------------------------------------------------
# Host-side preprocessing
# ----------------------------------------------------------------------------

def _wrap16(idx_flat):
    """dma_gather index layout: [128, n/16] int16, idx i at [i%16, i//16],
    replicated across the 8 groups of 16 partitions."""
    n = idx_flat.shape[0]
    assert n % 16 == 0
    w = idx_flat.reshape(n // 16, 16).T.astype(np.int16)  # [16, n/16]
    return np.tile(w, (8, 1))  # [128, n/16]


def _row_width(H, C):
    """h_aug row width: H*C features + 2H scores, padded so the bf16 row is
    a multiple of 256 bytes (dma_gather elem_size constraint)."""
    used = H * C + 2 * H
    return ((used + 127) // 128) * 128


def prep_static(edge_index, N, NPAD):
    """Degree-sorted node permutation + slot-major gather tables.

    Returns (Dpad, idx_cores, node_of_row, PAD_P0)."""
    loops = np.arange(N, dtype=np.int64)
    src = np.concatenate([edge_index[0].astype(np.int64), loops])
    dst = np.concatenate([edge_index[1].astype(np.int64), loops])
    deg = np.bincount(dst, minlength=NPAD)  # pad nodes have degree 0
    order = np.argsort(-deg, kind="stable")

    BPC = NPAD // (128 * NCORES)
    NPC = NPAD // NCORES
    node_of_row = np.empty(NPAD, dtype=np.int64)
    for b in range(BPC - 1):
        chunk = order[b * 1024:(b + 1) * 1024]
        q = np.arange(1024)
        rows = (q // 128) * NPC + b * 128 + (q % 128)
        node_of_row[rows] = chunk
    # last chunk: reals first on every core, pads fill the tail partitions
    last = order[(BPC - 1) * 1024:]
    n_real = int((deg[last] > 0).sum())
    assert n_real % NCORES == 0
    reals, pads = last[:n_real], last[n_real:]
    rpc = n_real // NCORES
    ppc = (1024 - n_real) // NCORES
    b = BPC - 1
    for c in range(NCORES):
        base = c * NPC + b * 128
        node_of_row[base:base + rpc] = reals[c * rpc:(c + 1) * rpc]
        node_of_row[base + rpc:base + 128] = pads[c * ppc:(c + 1) * ppc]
    row_of_node = np.empty(NPAD, dtype=np.int64)
    row_of_node[node_of_row] = np.arange(NPAD)
    PAD_P0 = rpc
    PADROW = NPAD  # dedicated poison row appended past the node table

    Dpad = []
    for b in range(BPC):
        mx = int(deg[order[b * 1024]])
        Dpad.append(max(GC, ((mx + GC - 1) // GC) * GC))

    sidx = np.argsort(dst, kind="stable")
    src_s, dst_s = src[sidx], dst[sidx]
    starts = np.searchsorted(dst_s, np.arange(NPAD))
    ends = np.searchsorted(dst_s, np.arange(NPAD) + 1)

    idx_cores = []
    for c in range(NCORES):
        cols = []
        for b in range(BPC):
            D = Dpad[b]
            flat = np.full(D * 128, PADROW, dtype=np.int64)
            for p in range(128):
                v = node_of_row[c * NPC + b * 128 + p]
                s0, s1 = starts[v], ends[v]
                k = s1 - s0
                if k:
                    flat[np.arange(k) * 128 + p] = row_of_node[src_s[s0:s1]]
            cols.append(_wrap16(flat))
        idx_cores.append(np.ascontiguousarray(np.concatenate(cols, axis=1)))
    return tuple(Dpad), idx_cores, node_of_row, PAD_P0


def prep_values(x, Ws, a_srcs, a_dsts, NPAD, node_of_row):
    N, F = x.shape
    xp = np.zeros((NPAD, F), dtype=np.float32)
    xp[:N] = x
    xperm = xp[node_of_row]  # row r holds node node_of_row[r]
    xT = np.ascontiguousarray(xperm.T).astype(BF16)  # [F, NPAD]

    W_augs = []
    for W, a_s, a_d in zip(Ws, a_srcs, a_dsts):
        H, Fin, C = W.shape
        RW = _row_width(H, C)
        Wf = np.transpose(W, (1, 0, 2)).reshape(Fin, H * C)
        wsrc = np.einsum("hfc,hc->fh", W, a_s)
        wdst = np.einsum("hfc,hc->fh", W, a_d)
        Wa = np.zeros((Fin, RW), dtype=np.float32)
        Wa[:, :H * C] = Wf
        Wa[:, H * C:H * C + H] = wsrc
        Wa[:, H * C + H:H * C + 2 * H] = wdst
        W_augs.append(Wa.astype(BF16))
    return xT, W_augs


# ----------------------------------------------------------------------------
# Device program
# ----------------------------------------------------------------------------

def build_nc(cfg, repeat=1):
    import concourse.bacc as bacc
    import concourse.mybir as mybir
    import concourse.tile as tile
    from contextlib import ExitStack

    f32 = mybir.dt.float32
    bf16 = mybir.dt.bfloat16
    i16 = mybir.dt.int16
    ALU = mybir.AluOpType
    ACT = mybir.ActivationFunctionType
    AX = mybir.AxisListType

    N = cfg["N"]
    NPAD = cfg["NPAD"]
    F_IN = cfg["F_IN"]
    C = cfg["C"]
    Dpad = cfg["Dpad"]
    HS = cfg["HS"]
    PAD_P0 = cfg["PAD_P0"]
    BPC = NPAD // (128 * NCORES)
    NPC = NPAD // NCORES
    NL = len(HS)
    RWs = [_row_width(H, C) for H in HS]
    FINs = [F_IN] + [C] * (NL - 1)
    DSUM = sum(Dpad)
    doff = [0]
    for d in Dpad:
        doff.append(doff[-1] + d)
    Dmax = max(Dpad)
    G1W = Dmax * max(RWs)

    nc = bacc.Bacc("TRN2", target_bir_lowering=False, debug=False,
                   num_devices=NCORES)

    xT_d = nc.dram_tensor("xT", [F_IN, NPC], bf16, kind="ExternalInput")
    idx_d = nc.dram_tensor("idxs", [128, DSUM * 8], i16, kind="ExternalInput")
    idt_d = nc.dram_tensor("idt", [128, NPC // 16], i16, kind="ExternalInput")
    W_d = [nc.dram_tensor(f"w{i+1}", [FINs[i], RWs[i]], bf16,
                          kind="ExternalInput") for i in range(NL)]
    bb_d = [nc.dram_tensor(f"bb{i+1}", [128, C], f32, kind="ExternalInput")
            for i in range(NL - 1)]
    b3_d = nc.dram_tensor("b3r", [1, C], f32, kind="ExternalInput")
    out_d = nc.dram_tensor("out", [1, C], f32, kind="ExternalOutput")

    with tile.TileContext(nc, num_cores=NCORES) as tc, ExitStack() as ctx:
        dram = ctx.enter_context(tc.tile_pool(name="dram", bufs=1, space="DRAM"))
        cpool = ctx.enter_context(tc.tile_pool(name="consts", bufs=1))
        hpool = ctx.enter_context(tc.tile_pool(name="hs", bufs=2))
        wpool = ctx.enter_context(tc.tile_pool(name="work", bufs=2))
        psum = ctx.enter_context(tc.tile_pool(name="ps", bufs=2, space="PSUM"))

        hl = [dram.tile([NPC, RWs[i]], bf16, tag=f"hl{i}", name=f"hl{i}")
              for i in range(NL)]
        # one extra row past the node table: the poison row pad slots point at
        hf = [dram.tile([NPAD + 1, RWs[i]], bf16, tag=f"hf{i}", name=f"hf{i}")
              for i in range(NL)]
        x2d = [dram.tile([NPC, C], bf16, tag=f"x2d{i}", name=f"x2d{i}")
               for i in range(NL - 1)]

        xT_sb = cpool.tile([F_IN, NPC], bf16, tag="xT")
        nc.sync.dma_start(xT_sb[:], xT_d[:, :])
        idx_sb = cpool.tile([128, DSUM * 8], i16, tag="idx")
        nc.sync.dma_start(idx_sb[:], idx_d[:, :])
        idt_sb = cpool.tile([128, NPC // 16], i16, tag="idt")
        nc.sync.dma_start(idt_sb[:], idt_d[:, :])
        W_sb = []
        for i in range(NL):
            w = cpool.tile([FINs[i], RWs[i]], bf16, tag=f"w{i}", name=f"w{i}")
            nc.sync.dma_start(w[:], W_d[i][:, :])
            W_sb.append(w)
        bb_sb = []
        for i in range(NL - 1):
            t = cpool.tile([128, C], f32, tag=f"bb{i}", name=f"bb{i}")
            nc.sync.dma_start(t[:], bb_d[i][:, :])
            bb_sb.append(t)
        b3_sb = cpool.tile([1, C], f32, tag="b3")
        nc.sync.dma_start(b3_sb[:], b3_d[:, :])
        ones_sb = cpool.tile([128, 1], f32, tag="ones")
        nc.vector.memset(ones_sb[:], 1.0)
        pois_sb = cpool.tile([1, max(RWs)], bf16, tag="pois")
        nc.vector.memset(pois_sb[:], -1e9)
        x2T = [cpool.tile([C, NPC], bf16, tag=f"x2T{i}", name=f"x2T{i}")
               for i in range(NL - 1)]
        g1f = cpool.tile([128, G1W], bf16, tag="g1f")
        msgf = cpool.tile([128, max(HS) * C * Dmax], bf16, tag="msgf")

        pfin = psum.tile([1, C], f32, tag="pfin", bufs=1)

        for _rep in range(repeat):
         for L in range(NL):
            H = HS[L]
            RW = RWs[L]
            SOFF = H * C

            # ---- phase A: h_aug for own nodes ----
            if L > 0:
                # transposed activations via DMA XBAR transpose
                nc.sync.dma_start(x2T[L - 1][:, :], x2d[L - 1][:, :],
                                  transpose=True)
            lsrc = xT_sb if L == 0 else x2T[L - 1]
            hs4 = None
            for b in range(BPC):
                j = b % 4
                if j == 0:
                    hs4 = hpool.tile([128, 4, RW], bf16, tag="hs4")
                lhsT = lsrc[:, b * 128:(b + 1) * 128]
                if RW > 512:
                    pA = psum.tile([128, 512], f32, tag="pA")
                    nc.tensor.matmul(pA[:], lhsT, W_sb[L][:, 0:512],
                                     start=True, stop=True)
                    pB = psum.tile([128, RW - 512], f32, tag="pB")
                    nc.tensor.matmul(pB[:], lhsT, W_sb[L][:, 512:RW],
                                     start=True, stop=True)
                    nc.scalar.copy(hs4[:, j, 0:512], pA[:])
                    nc.vector.tensor_copy(hs4[:, j, 512:RW], pB[:])
                else:
                    pA = psum.tile([128, RW], f32, tag="pA")
                    nc.tensor.matmul(pA[:], lhsT, W_sb[L][:, 0:RW],
                                     start=True, stop=True)
                    nc.scalar.copy(hs4[:, j, :], pA[:])
                if j == 3:
                    dst_ap = hl[L][(b - 3) * 128:(b + 1) * 128, :].rearrange(
                        "(j p) w -> p j w", j=4)
                    nc.sync.dma_start(dst_ap, hs4[:])

            # ---- phase B: allgather + poison row ----
            nc.gpsimd.collective_compute(
                "AllGather", mybir.AluOpType.bypass,
                replica_groups=[list(range(NCORES))],
                ins=[hl[L][:, :].opt()],
                outs=[hf[L][0:NPAD, :].opt()],
            )
            nc.sync.dma_start(hf[L][NPAD:NPAD + 1, :], pois_sb[:, 0:RW])
            # own nodes' s_dst, [128, BPC, H]
            sdl = wpool.tile([128, BPC, H], bf16, tag="sdl")
            hl_ap = hl[L][:, :].rearrange("(b p) w -> p b w", p=128)
            nc.sync.dma_start(sdl[:], hl_ap[:, :, SOFF + H:SOFF + 2 * H])

            # ---- phase C: per dst block ----
            for b in range(BPC):
                D = Dpad[b]
                g1 = g1f[:].rearrange("p (d w) -> p d w", w=RW)[:, 0:D, :]
                nc.gpsimd.dma_gather(
                    g1[:, :, :], hf[L][:, :],
                    idx_sb[:, doff[b] * 8:(doff[b] + D) * 8],
                    D * 128, D * 128, RW, single_packet=False)
                sc = wpool.tile([128, D, H], f32, tag="sc")
                nc.vector.tensor_tensor(
                    sc[:], g1[:, :, SOFF:SOFF + H],
                    sdl[:, b:b + 1, :].broadcast_to([128, D, H]), ALU.add)
                lr = wpool.tile([128, D, H], f32, tag="lr")
                nc.vector.scalar_tensor_tensor(lr[:], sc[:], 0.2, sc[:],
                                               op0=ALU.mult, op1=ALU.max)
                p = wpool.tile([128, D, H], f32, tag="p")
                nc.scalar.activation(p[:], lr[:], ACT.Exp)

                msg = msgf[:, 0:H * C * D].rearrange(
                    "p (h c d) -> p h c d", h=H, c=C)
                g1v = g1[:, :, 0:SOFF].rearrange("p d (h c) -> p h c d", h=H)
                pv = p[:].rearrange("p d h -> p h d").unsqueeze(2).broadcast_to(
                    [128, H, C, D])
                nc.vector.tensor_tensor(msg, g1v, pv, ALU.mult)
                nm = wpool.tile([128, H, C], f32, tag="nm")
                nc.vector.tensor_reduce(nm[:], msg, axis=AX.X, op=ALU.add)
                dn0 = wpool.tile([128, H], f32, tag="dn0")
                nc.vector.tensor_reduce(dn0[:], p[:].rearrange("p d h -> p h d"),
                                        axis=AX.X, op=ALU.add)
                dn = wpool.tile([128, H], f32, tag="dn")
                nc.vector.tensor_scalar(dn[:], dn0[:], float(H), 1e-16 * H,
                                        op0=ALU.mult, op1=ALU.add)
                rc = wpool.tile([128, H], f32, tag="rc")
                nc.vector.reciprocal(rc[:], dn[:])
                if L < NL - 1:
                    nm2 = wpool.tile([128, H, C], f32, tag="nm2")
                    nc.vector.tensor_tensor(
                        nm2[:], nm[:],
                        rc[:].unsqueeze(2).broadcast_to([128, H, C]), ALU.mult)
                    xo = wpool.tile([128, C], f32, tag="xo")
                    nc.vector.tensor_reduce(xo[:],
                                            nm2[:].rearrange("p h c -> p c h"),
                                            axis=AX.X, op=ALU.add)
                    xb = wpool.tile([128, C], f32, tag="xb")
                    nc.vector.tensor_tensor(xb[:], xo[:], bb_sb[L][:, :],
                                            ALU.add)
                    x2b = wpool.tile([128, C], bf16, tag="x2b")
                    nc.vector.tensor_scalar(x2b[:], xb[:], 0.0, None,
                                            op0=ALU.max)
                    nc.sync.dma_start(x2d[L][b * 128:(b + 1) * 128, :], x2b[:])
                else:
                    o3 = wpool.tile([128, C], f32, tag="o3")
                    nc.vector.tensor_scalar(o3[:], nm[:, 0, :], rc[:, 0:1],
                                            None, op0=ALU.mult)
                    nc.tensor.matmul(pfin[:], ones_sb[:], o3[:],
                                     start=(b == 0), stop=(b == BPC - 1))

        fs = wpool.tile([1, C], f32, tag="fs")
        nc.vector.tensor_scalar(fs[:], pfin[:], 1.0 / N, None, op0=ALU.mult)
        fs2 = wpool.tile([1, C], f32, tag="fs2")
        nc.vector.tensor_tensor(fs2[:], fs[:], b3_sb[:], ALU.add)
        nc.sync.dma_start(out_d[:, :], fs2[:])

    nc.compile()
    return nc


# ----------------------------------------------------------------------------
# Entry points
# ----------------------------------------------------------------------------

_PREP_CACHE = {}


def make_cfg_and_maps(inputs):
    x = np.asarray(inputs["x"])
    edge_index = np.asarray(inputs["edge_index"])
    N, F_IN = x.shape
    NPAD = ((N + 1023) // 1024) * 1024
    Ws = [np.asarray(inputs[f"W{i}"]) for i in (1, 2, 3)]
    a_srcs = [np.asarray(inputs[f"as{i}"]) for i in (1, 2, 3)]
    a_dsts = [np.asarray(inputs[f"ad{i}"]) for i in (1, 2, 3)]
    bs = [np.asarray(inputs[f"b{i}"]) for i in (1, 2, 3)]
    HS = tuple(W.shape[0] for W in Ws)
    C = Ws[0].shape[2]

    pkey = (id(inputs["edge_index"]), edge_index.shape, N)
    if pkey not in _PREP_CACHE:
        _PREP_CACHE.clear()
        _PREP_CACHE[pkey] = prep_static(edge_index, N, NPAD)
    Dpad, idx_cores, node_of_row, PAD_P0 = _PREP_CACHE[pkey]
    xT, W_augs = prep_values(x, Ws, a_srcs, a_dsts, NPAD, node_of_row)

    cfg = dict(N=N, NPAD=NPAD, F_IN=F_IN, C=C, Dpad=Dpad, HS=HS,
               PAD_P0=PAD_P0)
    NPC = NPAD // NCORES
    idt = _wrap16(np.arange(NPC, dtype=np.int64))
    in_maps = []
    for c in range(NCORES):
        m = {
            "xT": np.ascontiguousarray(xT[:, c * NPC:(c + 1) * NPC]),
            "idxs": idx_cores[c],
            "idt": idt,
            "b3r": (bs[2] * (1.0 / NCORES)).reshape(1, C).astype(np.float32),
        }
        for i in range(3):
            m[f"w{i+1}"] = W_augs[i]
        for i in range(2):
            m[f"bb{i+1}"] = np.ascontiguousarray(
                np.broadcast_to(bs[i].astype(np.float32)[None, :], (128, C)))
        in_maps.append(m)
    return cfg, in_maps


_NC_CACHE = {}


def _get_nc(cfg, repeat=1):
    key = (repeat,) + tuple(sorted((k, v) for k, v in cfg.items()))
    if key not in _NC_CACHE:
        _NC_CACHE[key] = build_nc(cfg, repeat=repeat)
    return _NC_CACHE[key]


def run(inputs, trace=False, repeat=1, **kw):
    from concourse.bass_utils import run_bass_kernel_spmd
    cfg, in_maps = make_cfg_and_maps(inputs)
    nc = _get_nc(cfg, repeat=repeat)
    res = run_bass_kernel_spmd(nc, in_maps, core_ids=list(range(NCORES)),
                               trace=trace, **kw)
    out = np.zeros((1, cfg["C"]), dtype=np.float32)
    for r in res.results:
        out += r["out"]
    return out, res


def kernel(**inputs) -> np.ndarray:
    out, _ = run(inputs)
    return out


# revision 15
# speedup vs baseline: 19.4697x; 1.0200x over previous
"""GAT (3-layer, PyG GATConv-style) Trainium2 Bass kernel, 8-core SPMD.

Strategy (degree-bucketed dst-major fixed-degree layout):
  - Nodes are permuted by in-degree (desc) and assigned to (core, block,
    partition): chunk b of 1024 sorted nodes -> block b on every core.
    Per-block slot count D_b = max in-degree within the chunk (padded to a
    multiple of 8), so padding waste stays ~15%.
  - Per layer: each core computes h_aug = x @ W_aug for its 2560 nodes
    (W_aug fuses per-head a_src/a_dst projections as trailing columns),
    stores to DRAM, one AllGather -> full node table hf.
  - Phase C per block of 128 dst nodes: dma_gather pulls the D_b incident
    src rows per dst into [128 dst, D_b, RW] (slot-major index tables), then
    a handful of giant DVE ops do the whole block: p = exp(leakyrelu(
    s_src + s_dst)), numer = reduce_d(p * h), denom = reduce_d(p),
    out = head_mean(numer / denom).  Padding slots point at a poisoned row
    (s_src = -1e9 -> p = 0), so no masking is needed.
  - Layer boundary: out blocks stored node-major to DRAM; the next layer's
    transposed activations are re-loaded via dma_gather(transpose=True)
    with an identity index table.
  - Layer 3 ends with a ones-vector matmul accumulating the node-sum
    partial; host sums the 8 per-core [1,128] partials.
"""

import numpy as np
import ml_dtypes

BF16 = ml_dtypes.bfloat16
NCORES = 8
GC = 6  # slots per gather chunk (6*128 = 768 idxs = 48 desc/engine)


# ----------------------------------------------------------------------------
# Host-side preprocessing
# ----------------------------------------------------------------------------

def _wrap16(idx_flat):
    """dma_gather index layout: [128, n/16] int16, idx i at [i%16, i//16],
    replicated across the 8 groups of 16 partitions."""
    n = idx_flat.shape[0]
    assert n % 16 == 0
    w = idx_flat.reshape(n // 16, 16).T.astype(np.int16)  # [16, n/16]
    return np.tile(w, (8, 1))  # [128, n/16]


def _row_width(H, C):
    """h_aug row width: H*C features + 2H scores, padded so the bf16 row is
    a multiple of 256 bytes (dma_gather elem_size constraint)."""
    used = H * C + 2 * H
    return ((used + 127) // 128) * 128


def prep_static(edge_index, N, NPAD):
    """Degree-sorted node permutation + slot-major gather tables.

    Returns (Dpad, idx_cores, node_of_row, PAD_P0)."""
    loops = np.arange(N, dtype=np.int64)
    src = np.concatenate([edge_index[0].astype(np.int64), loops])
    dst = np.concatenate([edge_index[1].astype(np.int64), loops])
    deg = np.bincount(dst, minlength=NPAD)  # pad nodes have degree 0
    order = np.argsort(-deg, kind="stable")

    BPC = NPAD // (128 * NCORES)
    NPC = NPAD // NCORES
    node_of_row = np.empty(NPAD, dtype=np.int64)
    for b in range(BPC - 1):
        chunk = order[b * 1024:(b + 1) * 1024]
        q = np.arange(1024)
        rows = (q // 128) * NPC + b * 128 + (q % 128)
        node_of_row[rows] = chunk
    # last chunk: reals first on every core, pads fill the tail partitions
    last = order[(BPC - 1) * 1024:]
    n_real = int((deg[last] > 0).sum())
    assert n_real % NCORES == 0
    reals, pads = last[:n_real], last[n_real:]
    rpc = n_real // NCORES
    ppc = (1024 - n_real) // NCORES
    b = BPC - 1
    for c in range(NCORES):
        base = c * NPC + b * 128
        node_of_row[base:base + rpc] = reals[c * rpc:(c + 1) * rpc]
        node_of_row[base + rpc:base + 128] = pads[c * ppc:(c + 1) * ppc]
    row_of_node = np.empty(NPAD, dtype=np.int64)
    row_of_node[node_of_row] = np.arange(NPAD)
    PAD_P0 = rpc
    PADROW = NPAD  # dedicated poison row appended past the node table

    Dpad = []
    for b in range(BPC):
        mx = int(deg[order[b * 1024]])
        Dpad.append(max(1, mx))

    sidx = np.argsort(dst, kind="stable")
    src_s, dst_s = src[sidx], dst[sidx]
    starts = np.searchsorted(dst_s, np.arange(NPAD))
    ends = np.searchsorted(dst_s, np.arange(NPAD) + 1)

    idx_cores = []
    for c in range(NCORES):
        cols = []
        for b in range(BPC):
            D = Dpad[b]
            flat = np.full(D * 128, PADROW, dtype=np.int64)
            for p in range(128):
                v = node_of_row[c * NPC + b * 128 + p]
                s0, s1 = starts[v], ends[v]
                k = s1 - s0
                if k:
                    flat[np.arange(k) * 128 + p] = row_of_node[src_s[s0:s1]]
            cols.append(_wrap16(flat))
        idx_cores.append(np.ascontiguousarray(np.concatenate(cols, axis=1)))
    return tuple(Dpad), idx_cores, node_of_row, PAD_P0


def prep_values(x, Ws, a_srcs, a_dsts, NPAD, node_of_row):
    N, F = x.shape
    xp = np.zeros((NPAD, F), dtype=np.float32)
    xp[:N] = x
    xperm = xp[node_of_row]  # row r holds node node_of_row[r]
    xT = np.ascontiguousarray(xperm.T).astype(BF16)  # [F, NPAD]

    W_augs = []
    for W, a_s, a_d in zip(Ws, a_srcs, a_dsts):
        H, Fin, C = W.shape
        RW = _row_width(H, C)
        Wf = np.transpose(W, (1, 0, 2)).reshape(Fin, H * C)
        wsrc = np.einsum("hfc,hc->fh", W, a_s)
        wdst = np.einsum("hfc,hc->fh", W, a_d)
        Wa = np.zeros((Fin, RW), dtype=np.float32)
        Wa[:, :H * C] = Wf
        Wa[:, H * C:H * C + H] = wsrc
        Wa[:, H * C + H:H * C + 2 * H] = wdst
        W_augs.append(Wa.astype(BF16))
    return xT, W_augs


# ----------------------------------------------------------------------------
# Device program
# ----------------------------------------------------------------------------

def build_nc(cfg, repeat=1):
    import concourse.bacc as bacc
    import concourse.mybir as mybir
    import concourse.tile as tile
    from contextlib import ExitStack

    f32 = mybir.dt.float32
    bf16 = mybir.dt.bfloat16
    i16 = mybir.dt.int16
    ALU = mybir.AluOpType
    ACT = mybir.ActivationFunctionType
    AX = mybir.AxisListType

    N = cfg["N"]
    NPAD = cfg["NPAD"]
    F_IN = cfg["F_IN"]
    C = cfg["C"]
    Dpad = cfg["Dpad"]
    HS = cfg["HS"]
    PAD_P0 = cfg["PAD_P0"]
    BPC = NPAD // (128 * NCORES)
    NPC = NPAD // NCORES
    NL = len(HS)
    RWs = [_row_width(H, C) for H in HS]
    FINs = [F_IN] + [C] * (NL - 1)
    DSUM = sum(Dpad)
    doff = [0]
    for d in Dpad:
        doff.append(doff[-1] + d)
    Dmax = max(Dpad)
    G1W = Dmax * max(RWs)

    nc = bacc.Bacc("TRN2", target_bir_lowering=False, debug=False,
                   num_devices=NCORES)

    xT_d = nc.dram_tensor("xT", [F_IN, NPC], bf16, kind="ExternalInput")
    idx_d = nc.dram_tensor("idxs", [128, DSUM * 8], i16, kind="ExternalInput")
    W_d = [nc.dram_tensor(f"w{i+1}", [FINs[i], RWs[i]], bf16,
                          kind="ExternalInput") for i in range(NL)]
    bb_d = [nc.dram_tensor(f"bb{i+1}", [128, C], f32, kind="ExternalInput")
            for i in range(NL - 1)]
    b3_d = nc.dram_tensor("b3r", [1, C], f32, kind="ExternalInput")
    out_d = nc.dram_tensor("out", [1, C], f32, kind="ExternalOutput")

    with tile.TileContext(nc, num_cores=NCORES) as tc, ExitStack() as ctx:
        dram = ctx.enter_context(tc.tile_pool(name="dram", bufs=1, space="DRAM"))
        cpool = ctx.enter_context(tc.tile_pool(name="consts", bufs=1))
        hpool = ctx.enter_context(tc.tile_pool(name="hs", bufs=2))
        wpool = ctx.enter_context(tc.tile_pool(name="work", bufs=2))
        psum = ctx.enter_context(tc.tile_pool(name="ps", bufs=2, space="PSUM"))

        hl = [dram.tile([NPC, RWs[i]], bf16, tag=f"hl{i}", name=f"hl{i}")
              for i in range(NL)]
        # one extra row past the node table: the poison row pad slots point at
        hf = [dram.tile([NPAD + 1, RWs[i]], bf16, tag=f"hf{i}", name=f"hf{i}")
              for i in range(NL)]
        x2d = [dram.tile([NPC, C], bf16, tag=f"x2d{i}", name=f"x2d{i}")
               for i in range(NL - 1)]

        xT_sb = cpool.tile([F_IN, NPC], bf16, tag="xT")
        nc.sync.dma_start(xT_sb[:], xT_d[:, :])
        idx_sb = cpool.tile([128, DSUM * 8], i16, tag="idx")
        nc.sync.dma_start(idx_sb[:], idx_d[:, :])
        W_sb = []
        for i in range(NL):
            w = cpool.tile([FINs[i], RWs[i]], bf16, tag=f"w{i}", name=f"w{i}")
            nc.sync.dma_start(w[:], W_d[i][:, :])
            W_sb.append(w)
        bb_sb = []
        for i in range(NL - 1):
            t = cpool.tile([128, C], f32, tag=f"bb{i}", name=f"bb{i}")
            nc.sync.dma_start(t[:], bb_d[i][:, :])
            bb_sb.append(t)
        b3_sb = cpool.tile([1, C], f32, tag="b3")
        nc.sync.dma_start(b3_sb[:], b3_d[:, :])
        ones_sb = cpool.tile([128, 1], f32, tag="ones")
        nc.vector.memset(ones_sb[:], 1.0)
        pois_sb = cpool.tile([1, max(RWs)], bf16, tag="pois")
        nc.vector.memset(pois_sb[:], -1e9)
        x2T = [cpool.tile([C, NPC], bf16, tag=f"x2T{i}", name=f"x2T{i}")
               for i in range(NL - 1)]
        g1f = cpool.tile([128, G1W], bf16, tag="g1f")
        msgf = cpool.tile([128, max(HS) * C * Dmax], bf16, tag="msgf")

        pfin = psum.tile([1, C], f32, tag="pfin", bufs=1)

        for _rep in range(repeat):
         for L in range(NL):
            H = HS[L]
            RW = RWs[L]
            SOFF = H * C

            # ---- phase A: h_aug for own nodes ----
            if L > 0:
                # transposed activations via DMA XBAR transpose
                nc.sync.dma_start(x2T[L - 1][:, :], x2d[L - 1][:, :],
                                  transpose=True)
            lsrc = xT_sb if L == 0 else x2T[L - 1]
            hs4 = None
            for b in range(BPC):
                j = b % 4
                if j == 0:
                    hs4 = hpool.tile([128, 4, RW], bf16, tag="hs4")
                lhsT = lsrc[:, b * 128:(b + 1) * 128]
                if RW > 512:
                    pA = psum.tile([128, 512], f32, tag="pA")
                    nc.tensor.matmul(pA[:], lhsT, W_sb[L][:, 0:512],
                                     start=True, stop=True)
                    pB = psum.tile([128, RW - 512], f32, tag="pB")
                    nc.tensor.matmul(pB[:], lhsT, W_sb[L][:, 512:RW],
                                     start=True, stop=True)
                    nc.scalar.copy(hs4[:, j, 0:512], pA[:])
                    nc.vector.tensor_copy(hs4[:, j, 512:RW], pB[:])
                else:
                    pA = psum.tile([128, RW], f32, tag="pA")
                    nc.tensor.matmul(pA[:], lhsT, W_sb[L][:, 0:RW],
                                     start=True, stop=True)
                    nc.scalar.copy(hs4[:, j, :], pA[:])
                if j == 3:
                    dst_ap = hl[L][(b - 3) * 128:(b + 1) * 128, :].rearrange(
                        "(j p) w -> p j w", j=4)
                    nc.sync.dma_start(dst_ap, hs4[:])

            # ---- phase B: allgather + poison row ----
            nc.gpsimd.collective_compute(
                "AllGather", mybir.AluOpType.bypass,
                replica_groups=[list(range(NCORES))],
                ins=[hl[L][:, :].opt()],
                outs=[hf[L][0:NPAD, :].opt()],
            )
            nc.sync.dma_start(hf[L][NPAD:NPAD + 1, :], pois_sb[:, 0:RW])
            # own nodes' s_dst, [128, BPC, H]
            sdl = wpool.tile([128, BPC, H], bf16, tag="sdl")
            hl_ap = hl[L][:, :].rearrange("(b p) w -> p b w", p=128)
            nc.sync.dma_start(sdl[:], hl_ap[:, :, SOFF + H:SOFF + 2 * H])

            # ---- phase C: per dst block ----
            for b in range(BPC):
                D = Dpad[b]
                g1 = g1f[:].rearrange("p (d w) -> p d w", w=RW)[:, 0:D, :]
                nc.gpsimd.dma_gather(
                    g1[:, :, :], hf[L][:, :],
                    idx_sb[:, doff[b] * 8:(doff[b] + D) * 8],
                    D * 128, D * 128, RW, single_packet=False)
                sc = wpool.tile([128, D, H], f32, tag="sc")
                nc.vector.tensor_tensor(
                    sc[:], g1[:, :, SOFF:SOFF + H],
                    sdl[:, b:b + 1, :].broadcast_to([128, D, H]), ALU.add)
                lr = wpool.tile([128, D, H], f32, tag="lr")
                nc.vector.scalar_tensor_tensor(lr[:], sc[:], 0.2, sc[:],
                                               op0=ALU.mult, op1=ALU.max)
                p = wpool.tile([128, D, H], f32, tag="p")
                nc.scalar.activation(p[:], lr[:], ACT.Exp)

                msg = msgf[:, 0:H * C * D].rearrange(
                    "p (h c d) -> p h c d", h=H, c=C)
                g1v = g1[:, :, 0:SOFF].rearrange("p d (h c) -> p h c d", h=H)
                pv = p[:].rearrange("p d h -> p h d").unsqueeze(2).broadcast_to(
                    [128, H, C, D])
                nc.vector.tensor_tensor(msg, g1v, pv, ALU.mult)
                nm = wpool.tile([128, H, C], f32, tag="nm")
                nc.vector.tensor_reduce(nm[:], msg, axis=AX.X, op=ALU.add)
                dn0 = wpool.tile([128, H], f32, tag="dn0")
                nc.vector.tensor_reduce(dn0[:], p[:].rearrange("p d h -> p h d"),
                                        axis=AX.X, op=ALU.add)
                dn = wpool.tile([128, H], f32, tag="dn")
                nc.vector.tensor_scalar(dn[:], dn0[:], float(H), 1e-16 * H,
                                        op0=ALU.mult, op1=ALU.add)
                rc = wpool.tile([128, H], f32, tag="rc")
                nc.vector.reciprocal(rc[:], dn[:])
                if L < NL - 1:
                    nm2 = wpool.tile([128, H, C], f32, tag="nm2")
                    nc.vector.tensor_tensor(
                        nm2[:], nm[:],
                        rc[:].unsqueeze(2).broadcast_to([128, H, C]), ALU.mult)
                    xo = wpool.tile([128, C], f32, tag="xo")
                    nc.vector.tensor_reduce(xo[:],
                                            nm2[:].rearrange("p h c -> p c h"),
                                            axis=AX.X, op=ALU.add)
                    xb = wpool.tile([128, C], f32, tag="xb")
                    nc.vector.tensor_tensor(xb[:], xo[:], bb_sb[L][:, :],
                                            ALU.add)
                    x2b = wpool.tile([128, C], bf16, tag="x2b")
                    nc.vector.tensor_scalar(x2b[:], xb[:], 0.0, None,
                                            op0=ALU.max)
                    nc.sync.dma_start(x2d[L][b * 128:(b + 1) * 128, :], x2b[:])
                else:
                    o3 = wpool.tile([128, C], f32, tag="o3")
                    nc.vector.tensor_scalar(o3[:], nm[:, 0, :], rc[:, 0:1],
                                            None, op0=ALU.mult)
                    nc.tensor.matmul(pfin[:], ones_sb[:], o3[:],
                                     start=(b == 0), stop=(b == BPC - 1))

        fs = wpool.tile([1, C], f32, tag="fs")
        nc.vector.tensor_scalar(fs[:], pfin[:], 1.0 / N, None, op0=ALU.mult)
        fs2 = wpool.tile([1, C], f32, tag="fs2")
        nc.vector.tensor_tensor(fs2[:], fs[:], b3_sb[:], ALU.add)
        nc.sync.dma_start(out_d[:, :], fs2[:])

    nc.compile()
    return nc


# ----------------------------------------------------------------------------
# Entry points
# ----------------------------------------------------------------------------

_PREP_CACHE = {}


def make_cfg_and_maps(inputs):
    x = np.asarray(inputs["x"])
    edge_index = np.asarray(inputs["edge_index"])
    N, F_IN = x.shape
    NPAD = ((N + 1023) // 1024) * 1024
    Ws = [np.asarray(inputs[f"W{i}"]) for i in (1, 2, 3)]
    a_srcs = [np.asarray(inputs[f"as{i}"]) for i in (1, 2, 3)]
    a_dsts = [np.asarray(inputs[f"ad{i}"]) for i in (1, 2, 3)]
    bs = [np.asarray(inputs[f"b{i}"]) for i in (1, 2, 3)]
    HS = tuple(W.shape[0] for W in Ws)
    C = Ws[0].shape[2]

    pkey = (id(inputs["edge_index"]), edge_index.shape, N)
    if pkey not in _PREP_CACHE:
        _PREP_CACHE.clear()
        _PREP_CACHE[pkey] = prep_static(edge_index, N, NPAD)
    Dpad, idx_cores, node_of_row, PAD_P0 = _PREP_CACHE[pkey]
    xT, W_augs = prep_values(x, Ws, a_srcs, a_dsts, NPAD, node_of_row)

    cfg = dict(N=N, NPAD=NPAD, F_IN=F_IN, C=C, Dpad=Dpad, HS=HS,
               PAD_P0=PAD_P0)
    NPC = NPAD // NCORES
    in_maps = []
    for c in range(NCORES):
        m = {
            "xT": np.ascontiguousarray(xT[:, c * NPC:(c + 1) * NPC]),
            "idxs": idx_cores[c],
            "b3r": (bs[2] * (1.0 / NCORES)).reshape(1, C).astype(np.float32),
        }
        for i in range(3):
            m[f"w{i+1}"] = W_augs[i]
        for i in range(2):
            m[f"bb{i+1}"] = np.ascontiguousarray(
                np.broadcast_to(bs[i].astype(np.float32)[None, :], (128, C)))
        in_maps.append(m)
    return cfg, in_maps


_NC_CACHE = {}


def _get_nc(cfg, repeat=1):
    key = (repeat,) + tuple(sorted((k, v) for k, v in cfg.items()))
    if key not in _NC_CACHE:
        _NC_CACHE[key] = build_nc(cfg, repeat=repeat)
    return _NC_CACHE[key]


def run(inputs, trace=False, repeat=1, **kw):
    from concourse.bass_utils import run_bass_kernel_spmd
    cfg, in_maps = make_cfg_and_maps(inputs)
    nc = _get_nc(cfg, repeat=repeat)
    res = run_bass_kernel_spmd(nc, in_maps, core_ids=list(range(NCORES)),
                               trace=trace, **kw)
    out = np.zeros((1, cfg["C"]), dtype=np.float32)
    for r in res.results:
        out += r["out"]
    return out, res


def kernel(**inputs) -> np.ndarray:
    out, _ = run(inputs)
    return out


# revision 20
# speedup vs baseline: 34.8123x; 1.7880x over previous
"""GAT (3-layer, PyG GATConv-style) Trainium2 Bass kernel, 8-core SPMD.

Strategy (degree-bucketed dst-major fixed-degree layout):
  - Nodes are permuted by in-degree (desc) and assigned to (core, block,
    partition): chunk b of 1024 sorted nodes -> block b on every core.
    Per-block slot count D_b = max in-degree within the chunk (padded to a
    multiple of 8), so padding waste stays ~15%.
  - Per layer: each core computes h_aug = x @ W_aug for its 2560 nodes
    (W_aug fuses per-head a_src/a_dst projections as trailing columns),
    stores to DRAM, one AllGather -> full node table hf.
  - Phase C per block of 128 dst nodes: dma_gather pulls the D_b incident
    src rows per dst into [128 dst, D_b, RW] (slot-major index tables), then
    a handful of giant DVE ops do the whole block: p = exp(leakyrelu(
    s_src + s_dst)), numer = reduce_d(p * h), denom = reduce_d(p),
    out = head_mean(numer / denom).  Padding slots point at a poisoned row
    (s_src = -1e9 -> p = 0), so no masking is needed.
  - Layer boundary: out blocks stored node-major to DRAM; the next layer's
    transposed activations are re-loaded via dma_gather(transpose=True)
    with an identity index table.
  - Layer 3 ends with a ones-vector matmul accumulating the node-sum
    partial; host sums the 8 per-core [1,128] partials.
"""

import numpy as np
import ml_dtypes

BF16 = ml_dtypes.bfloat16
NCORES = 8
GC = 6  # slots per gather chunk (6*128 = 768 idxs = 48 desc/engine)


# ----------------------------------------------------------------------------
# Host-side preprocessing
# ----------------------------------------------------------------------------

def _wrap16(idx_flat):
    """dma_gather index layout: [128, n/16] int16, idx i at [i%16, i//16],
    replicated across the 8 groups of 16 partitions."""
    n = idx_flat.shape[0]
    assert n % 16 == 0
    w = idx_flat.reshape(n // 16, 16).T.astype(np.int16)  # [16, n/16]
    return np.tile(w, (8, 1))  # [128, n/16]


def _row_width(H, C):
    """h_aug row width: H*C features + 2H scores, padded so the bf16 row is
    a multiple of 256 bytes (dma_gather elem_size constraint)."""
    used = H * C + 2 * H
    return ((used + 127) // 128) * 128


def prep_static(edge_index, N, NPAD):
    """Degree-sorted node permutation + slot-major gather tables.

    Returns (Dpad, idx_cores, node_of_row, PAD_P0)."""
    loops = np.arange(N, dtype=np.int64)
    src = np.concatenate([edge_index[0].astype(np.int64), loops])
    dst = np.concatenate([edge_index[1].astype(np.int64), loops])
    deg = np.bincount(dst, minlength=NPAD)  # pad nodes have degree 0
    order = np.argsort(-deg, kind="stable")

    BPC = NPAD // (128 * NCORES)
    NPC = NPAD // NCORES
    node_of_row = np.empty(NPAD, dtype=np.int64)
    for b in range(BPC - 1):
        chunk = order[b * 1024:(b + 1) * 1024]
        q = np.arange(1024)
        rows = (q // 128) * NPC + b * 128 + (q % 128)
        node_of_row[rows] = chunk
    # last chunk: reals first on every core, pads fill the tail partitions
    last = order[(BPC - 1) * 1024:]
    n_real = int((deg[last] > 0).sum())
    assert n_real % NCORES == 0
    reals, pads = last[:n_real], last[n_real:]
    rpc = n_real // NCORES
    ppc = (1024 - n_real) // NCORES
    b = BPC - 1
    for c in range(NCORES):
        base = c * NPC + b * 128
        node_of_row[base:base + rpc] = reals[c * rpc:(c + 1) * rpc]
        node_of_row[base + rpc:base + 128] = pads[c * ppc:(c + 1) * ppc]
    row_of_node = np.empty(NPAD, dtype=np.int64)
    row_of_node[node_of_row] = np.arange(NPAD)
    PAD_P0 = rpc
    PADROW = NPAD  # dedicated poison row appended past the node table

    Dpad = []
    for b in range(BPC):
        mx = int(deg[order[b * 1024]])
        Dpad.append(max(1, mx))

    sidx = np.argsort(dst, kind="stable")
    src_s, dst_s = src[sidx], dst[sidx]
    starts = np.searchsorted(dst_s, np.arange(NPAD))
    ends = np.searchsorted(dst_s, np.arange(NPAD) + 1)

    idx_cores = []
    for c in range(NCORES):
        cols = []
        for b in range(BPC):
            D = Dpad[b]
            flat = np.full(D * 128, PADROW, dtype=np.int64)
            for p in range(128):
                v = node_of_row[c * NPC + b * 128 + p]
                s0, s1 = starts[v], ends[v]
                k = s1 - s0
                if k:
                    flat[np.arange(k) * 128 + p] = row_of_node[src_s[s0:s1]]
            cols.append(_wrap16(flat))
        idx_cores.append(np.ascontiguousarray(np.concatenate(cols, axis=1)))
    return tuple(Dpad), idx_cores, node_of_row, PAD_P0


def prep_values(x, Ws, a_srcs, a_dsts, NPAD, node_of_row):
    N, F = x.shape
    xp = np.zeros((NPAD, F), dtype=np.float32)
    xp[:N] = x
    xperm = xp[node_of_row]  # row r holds node node_of_row[r]
    xT = np.ascontiguousarray(xperm.T).astype(BF16)  # [F, NPAD]

    W_augs = []
    for W, a_s, a_d in zip(Ws, a_srcs, a_dsts):
        H, Fin, C = W.shape
        RW = _row_width(H, C)
        Wf = np.transpose(W, (1, 0, 2)).reshape(Fin, H * C)
        wsrc = np.einsum("hfc,hc->fh", W, a_s)
        wdst = np.einsum("hfc,hc->fh", W, a_d)
        Wa = np.zeros((Fin, RW), dtype=np.float32)
        Wa[:, :H * C] = Wf
        Wa[:, H * C:H * C + H] = wsrc
        Wa[:, H * C + H:H * C + 2 * H] = wdst
        W_augs.append(Wa.astype(BF16))
    return xT, W_augs


# ----------------------------------------------------------------------------
# Device program
# ----------------------------------------------------------------------------

def build_nc(cfg, repeat=1):
    import concourse.bacc as bacc
    import concourse.mybir as mybir
    import concourse.tile as tile
    from contextlib import ExitStack

    f32 = mybir.dt.float32
    bf16 = mybir.dt.bfloat16
    i16 = mybir.dt.int16
    ALU = mybir.AluOpType
    ACT = mybir.ActivationFunctionType
    AX = mybir.AxisListType

    N = cfg["N"]
    NPAD = cfg["NPAD"]
    F_IN = cfg["F_IN"]
    C = cfg["C"]
    Dpad = cfg["Dpad"]
    HS = cfg["HS"]
    PAD_P0 = cfg["PAD_P0"]
    BPC = NPAD // (128 * NCORES)
    NPC = NPAD // NCORES
    NL = len(HS)
    RWs = [_row_width(H, C) for H in HS]
    FINs = [F_IN] + [C] * (NL - 1)
    DSUM = sum(Dpad)
    doff = [0]
    for d in Dpad:
        doff.append(doff[-1] + d)
    Dmax = max(Dpad)
    G1W = Dmax * max(RWs)

    nc = bacc.Bacc("TRN2", target_bir_lowering=False, debug=False,
                   num_devices=NCORES)

    xT_d = nc.dram_tensor("xT", [F_IN, NPC], bf16, kind="ExternalInput")
    idx_d = nc.dram_tensor("idxs", [128, DSUM * 8], i16, kind="ExternalInput")
    W_d = [nc.dram_tensor(f"w{i+1}", [FINs[i], RWs[i]], bf16,
                          kind="ExternalInput") for i in range(NL)]
    bb_d = [nc.dram_tensor(f"bb{i+1}", [128, C], f32, kind="ExternalInput")
            for i in range(NL - 1)]
    b3_d = nc.dram_tensor("b3r", [1, C], f32, kind="ExternalInput")
    out_d = nc.dram_tensor("out", [1, C], f32, kind="ExternalOutput")

    with tile.TileContext(nc, num_cores=NCORES) as tc, ExitStack() as ctx:
        dram = ctx.enter_context(tc.tile_pool(name="dram", bufs=1, space="DRAM"))
        cpool = ctx.enter_context(tc.tile_pool(name="consts", bufs=1))
        hpool = ctx.enter_context(tc.tile_pool(name="hs", bufs=1))
        wpool = ctx.enter_context(tc.tile_pool(name="work", bufs=1))
        psum = ctx.enter_context(tc.tile_pool(name="ps", bufs=2, space="PSUM"))

        hl = [dram.tile([NPC, RWs[i]], bf16, tag=f"hl{i}", name=f"hl{i}")
              for i in range(NL)]
        # one extra row past the node table: the poison row pad slots point at
        hf = [dram.tile([NPAD + 1, RWs[i]], bf16, tag=f"hf{i}", name=f"hf{i}")
              for i in range(NL)]
        x2d = [dram.tile([NPC, C], bf16, tag=f"x2d{i}", name=f"x2d{i}")
               for i in range(NL - 1)]

        xT_sb = cpool.tile([F_IN, NPC], bf16, tag="xT")
        nc.sync.dma_start(xT_sb[:], xT_d[:, :])
        idx_sb = cpool.tile([128, DSUM * 8], i16, tag="idx")
        nc.sync.dma_start(idx_sb[:], idx_d[:, :])
        W_sb = []
        for i in range(NL):
            w = cpool.tile([FINs[i], RWs[i]], bf16, tag=f"w{i}", name=f"w{i}")
            nc.sync.dma_start(w[:], W_d[i][:, :])
            W_sb.append(w)
        bb_sb = []
        for i in range(NL - 1):
            t = cpool.tile([128, C], f32, tag=f"bb{i}", name=f"bb{i}")
            nc.sync.dma_start(t[:], bb_d[i][:, :])
            bb_sb.append(t)
        b3_sb = cpool.tile([1, C], f32, tag="b3")
        nc.sync.dma_start(b3_sb[:], b3_d[:, :])
        ones_sb = cpool.tile([128, 1], f32, tag="ones")
        nc.vector.memset(ones_sb[:], 1.0)
        pois_sb = cpool.tile([1, max(RWs)], bf16, tag="pois")
        nc.vector.memset(pois_sb[:], -1e9)
        x2T = [cpool.tile([C, NPC], bf16, tag=f"x2T{i}", name=f"x2T{i}")
               for i in range(NL - 1)]
        g1f = cpool.tile([128, G1W], bf16, tag="g1f")
        msgf = cpool.tile([128, max(HS) * C * Dmax], bf16, tag="msgf")

        pfin = psum.tile([1, C], f32, tag="pfin", bufs=1)

        for _rep in range(repeat):
         for L in range(NL):
            H = HS[L]
            RW = RWs[L]
            SOFF = H * C

            # ---- phase A: h_aug for own nodes ----
            if L > 0:
                # transposed activations via DMA XBAR transpose
                nc.sync.dma_start(x2T[L - 1][:, :], x2d[L - 1][:, :],
                                  transpose=True)
            lsrc = xT_sb if L == 0 else x2T[L - 1]
            for g0 in range(0, BPC, 4):
                hs4 = hpool.tile([128, 4, RW], bf16, tag="hs4")
                if RW > 512:
                    pA4 = psum.tile([128, 4, 512], f32, tag="pA4", bufs=1)
                    pB4 = psum.tile([128, 4, RW - 512], f32, tag="pB4", bufs=1)
                else:
                    pA4 = psum.tile([128, 4, RW], f32, tag="pA4", bufs=1)
                for j in range(4):
                    lhsT = lsrc[:, (g0 + j) * 128:(g0 + j + 1) * 128]
                    if RW > 512:
                        nc.tensor.matmul(pA4[:, j, :], lhsT, W_sb[L][:, 0:512],
                                         start=True, stop=True)
                        nc.tensor.matmul(pB4[:, j, :], lhsT, W_sb[L][:, 512:RW],
                                         start=True, stop=True)
                    else:
                        nc.tensor.matmul(pA4[:, j, :], lhsT, W_sb[L][:, 0:RW],
                                         start=True, stop=True)
                if RW > 512:
                    nc.scalar.copy(hs4[:, :, 0:512], pA4[:])
                    nc.vector.tensor_copy(hs4[:, :, 512:RW], pB4[:])
                else:
                    nc.scalar.copy(hs4[:], pA4[:])
                dst_ap = hl[L][g0 * 128:(g0 + 4) * 128, :].rearrange(
                    "(j p) w -> p j w", j=4)
                nc.sync.dma_start(dst_ap, hs4[:])

            # ---- phase B: allgather + poison row ----
            nc.gpsimd.collective_compute(
                "AllGather", mybir.AluOpType.bypass,
                replica_groups=[list(range(NCORES))],
                ins=[hl[L][:, :].opt()],
                outs=[hf[L][0:NPAD, :].opt()],
            )
            nc.sync.dma_start(hf[L][NPAD:NPAD + 1, :], pois_sb[:, 0:RW])
            # own nodes' s_dst, [128, BPC, H]
            sdl = wpool.tile([128, BPC, H], bf16, tag="sdl")
            hl_ap = hl[L][:, :].rearrange("(b p) w -> p b w", p=128)
            nc.sync.dma_start(sdl[:], hl_ap[:, :, SOFF + H:SOFF + 2 * H])

            # ---- phase C: per dst block (numer/denom), tail batched ----
            nma = wpool.tile([128, BPC, H, C], bf16, tag="nma")
            dn0a = wpool.tile([128, BPC, H], f32, tag="dn0a")
            for b in range(BPC):
                D = Dpad[b]
                g1 = g1f[:].rearrange("p (d w) -> p d w", w=RW)[:, 0:D, :]
                nc.gpsimd.dma_gather(
                    g1[:, :, :], hf[L][:, :],
                    idx_sb[:, doff[b] * 8:(doff[b] + D) * 8],
                    D * 128, D * 128, RW, single_packet=False)
                p = wpool.tile([128, D, H], f32, tag="p")
                sc = wpool.tile([128, D, H], f32, tag="sc")
                nc.vector.tensor_tensor(
                    sc[:], g1[:, :, SOFF:SOFF + H],
                    sdl[:, b:b + 1, :].broadcast_to([128, D, H]), ALU.add)
                lr = wpool.tile([128, D, H], f32, tag="lr")
                nc.vector.scalar_tensor_tensor(lr[:], sc[:], 0.2, sc[:],
                                               op0=ALU.mult, op1=ALU.max)
                nc.scalar.activation(p[:], lr[:], ACT.Exp)

                msg = msgf[:, 0:H * C * D].rearrange(
                    "p (h c d) -> p h c d", h=H, c=C)
                g1v = g1[:, :, 0:SOFF].rearrange("p d (h c) -> p h c d", h=H)
                pv = p[:].rearrange("p d h -> p h d").unsqueeze(2).broadcast_to(
                    [128, H, C, D])
                nc.vector.tensor_tensor(msg, g1v, pv, ALU.mult)
                with nc.allow_low_precision("bf16 numer accumulate"):
                    nc.vector.tensor_reduce(nma[:, b, :, :], msg, axis=AX.X,
                                            op=ALU.add)
                nc.vector.tensor_reduce(dn0a[:, b, :],
                                        p[:].rearrange("p d h -> p h d"),
                                        axis=AX.X, op=ALU.add)

            # ---- batched tail over all BPC blocks ----
            dna = wpool.tile([128, BPC, H], f32, tag="dna")
            nc.vector.tensor_scalar(dna[:], dn0a[:], float(H), 1e-16 * H,
                                    op0=ALU.mult, op1=ALU.add)
            rca = wpool.tile([128, BPC, H], f32, tag="rca")
            nc.vector.reciprocal(rca[:], dna[:])
            g1_f32 = g1f[:].bitcast(f32)
            if L < NL - 1:
                nm2a = msgf[:, 0:BPC * H * C].rearrange(
                    "p (b h c) -> p b h c", b=BPC, h=H)
                nc.vector.tensor_tensor(
                    nm2a, nma[:],
                    rca[:].unsqueeze(3).broadcast_to([128, BPC, H, C]),
                    ALU.mult)
                xoa = g1_f32[:, 0:BPC * C].rearrange("p (b c) -> p b c", b=BPC)
                nc.vector.tensor_reduce(xoa,
                                        nm2a.rearrange("p b h c -> p b c h"),
                                        axis=AX.X, op=ALU.add)
                xba = g1_f32[:, BPC * C:2 * BPC * C].rearrange(
                    "p (b c) -> p b c", b=BPC)
                nc.vector.tensor_tensor(
                    xba, xoa,
                    bb_sb[L][:].unsqueeze(1).broadcast_to([128, BPC, C]),
                    ALU.add)
                x2ba = g1f[:, 4 * BPC * C:5 * BPC * C].rearrange(
                    "p (b c) -> p b c", b=BPC)
                nc.vector.tensor_scalar(x2ba, xba, 0.0, None, op0=ALU.max)
                nc.sync.dma_start(
                    x2d[L][:, :].rearrange("(b p) c -> p b c", p=128), x2ba)
            else:
                o3a = g1_f32[:, 0:BPC * C].rearrange("p (b c) -> p b c", b=BPC)
                nc.vector.tensor_tensor(
                    o3a, nma[:, :, 0, :],
                    rca[:, :, 0:1].broadcast_to([128, BPC, C]), ALU.mult)
                o3s = wpool.tile([128, C], f32, tag="o3s")
                nc.vector.tensor_reduce(o3s[:],
                                        o3a.rearrange("p b c -> p c b"),
                                        axis=AX.X, op=ALU.add)
                nc.tensor.matmul(pfin[:], ones_sb[:], o3s[:],
                                 start=True, stop=True)

        fs = wpool.tile([1, C], f32, tag="fs")
        nc.vector.tensor_scalar(fs[:], pfin[:], 1.0 / N, None, op0=ALU.mult)
        fs2 = wpool.tile([1, C], f32, tag="fs2")
        nc.vector.tensor_tensor(fs2[:], fs[:], b3_sb[:], ALU.add)
        nc.sync.dma_start(out_d[:, :], fs2[:])

    nc.compile()
    return nc


# ----------------------------------------------------------------------------
# Entry points
# ----------------------------------------------------------------------------

_PREP_CACHE = {}


def make_cfg_and_maps(inputs):
    x = np.asarray(inputs["x"])
    edge_index = np.asarray(inputs["edge_index"])
    N, F_IN = x.shape
    NPAD = ((N + 1023) // 1024) * 1024
    Ws = [np.asarray(inputs[f"W{i}"]) for i in (1, 2, 3)]
    a_srcs = [np.asarray(inputs[f"as{i}"]) for i in (1, 2, 3)]
    a_dsts = [np.asarray(inputs[f"ad{i}"]) for i in (1, 2, 3)]
    bs = [np.asarray(inputs[f"b{i}"]) for i in (1, 2, 3)]
    HS = tuple(W.shape[0] for W in Ws)
    C = Ws[0].shape[2]

    pkey = (id(inputs["edge_index"]), edge_index.shape, N)
    if pkey not in _PREP_CACHE:
        _PREP_CACHE.clear()
        _PREP_CACHE[pkey] = prep_static(edge_index, N, NPAD)
    Dpad, idx_cores, node_of_row, PAD_P0 = _PREP_CACHE[pkey]
    xT, W_augs = prep_values(x, Ws, a_srcs, a_dsts, NPAD, node_of_row)

    cfg = dict(N=N, NPAD=NPAD, F_IN=F_IN, C=C, Dpad=Dpad, HS=HS,
               PAD_P0=PAD_P0)
    NPC = NPAD // NCORES
    in_maps = []
    for c in range(NCORES):
        m = {
            "xT": np.ascontiguousarray(xT[:, c * NPC:(c + 1) * NPC]),
            "idxs": idx_cores[c],
            "b3r": (bs[2] * (1.0 / NCORES)).reshape(1, C).astype(np.float32),
        }
        for i in range(3):
            m[f"w{i+1}"] = W_augs[i]
        for i in range(2):
            m[f"bb{i+1}"] = np.ascontiguousarray(
                np.broadcast_to(bs[i].astype(np.float32)[None, :], (128, C)))
        in_maps.append(m)
    return cfg, in_maps


_NC_CACHE = {}


def _get_nc(cfg, repeat=1):
    key = (repeat,) + tuple(sorted((k, v) for k, v in cfg.items()))
    if key not in _NC_CACHE:
        _NC_CACHE[key] = build_nc(cfg, repeat=repeat)
    return _NC_CACHE[key]


def run(inputs, trace=False, repeat=1, **kw):
    from concourse.bass_utils import run_bass_kernel_spmd
    cfg, in_maps = make_cfg_and_maps(inputs)
    nc = _get_nc(cfg, repeat=repeat)
    res = run_bass_kernel_spmd(nc, in_maps, core_ids=list(range(NCORES)),
                               trace=trace, **kw)
    out = np.zeros((1, cfg["C"]), dtype=np.float32)
    for r in res.results:
        out += r["out"]
    return out, res


def kernel(**inputs) -> np.ndarray:
    out, _ = run(inputs)
    return out


# revision 26
# speedup vs baseline: 41.7965x; 1.2006x over previous
"""GAT (3-layer, PyG GATConv-style) Trainium2 Bass kernel, 8-core SPMD.

Strategy (degree-bucketed dst-major fixed-degree layout):
  - Nodes are permuted by in-degree (desc) and assigned to (core, block,
    partition): chunk b of 1024 sorted nodes -> block b on every core.
    Per-block slot count D_b = max in-degree within the chunk (padded to a
    multiple of 8), so padding waste stays ~15%.
  - Per layer: each core computes h_aug = x @ W_aug for its 2560 nodes
    (W_aug fuses per-head a_src/a_dst projections as trailing columns),
    stores to DRAM, one AllGather -> full node table hf.
  - Phase C per block of 128 dst nodes: dma_gather pulls the D_b incident
    src rows per dst into [128 dst, D_b, RW] (slot-major index tables), then
    a handful of giant DVE ops do the whole block: p = exp(leakyrelu(
    s_src + s_dst)), numer = reduce_d(p * h), denom = reduce_d(p),
    out = head_mean(numer / denom).  Padding slots point at a poisoned row
    (s_src = -1e9 -> p = 0), so no masking is needed.
  - Layer boundary: out blocks stored node-major to DRAM; the next layer's
    transposed activations are re-loaded via dma_gather(transpose=True)
    with an identity index table.
  - Layer 3 ends with a ones-vector matmul accumulating the node-sum
    partial; host sums the 8 per-core [1,128] partials.
"""

import numpy as np
import ml_dtypes

BF16 = ml_dtypes.bfloat16
NCORES = 8
GC = 6  # slots per gather chunk (6*128 = 768 idxs = 48 desc/engine)


# ----------------------------------------------------------------------------
# Host-side preprocessing
# ----------------------------------------------------------------------------

def _wrap16(idx_flat):
    """dma_gather index layout: [128, n/16] int16, idx i at [i%16, i//16],
    replicated across the 8 groups of 16 partitions."""
    n = idx_flat.shape[0]
    assert n % 16 == 0
    w = idx_flat.reshape(n // 16, 16).T.astype(np.int16)  # [16, n/16]
    return np.tile(w, (8, 1))  # [128, n/16]


def _row_width(H, C):
    """h_aug row width: H*(C+1) features+ones + 2H scores, padded so the
    bf16 row is a multiple of 256 bytes (dma_gather elem_size constraint)."""
    used = H * (C + 1) + 2 * H
    return ((used + 127) // 128) * 128


def prep_static(edge_index, N, NPAD):
    """Degree-sorted node permutation + slot-major gather tables.

    Returns (Dpad, idx_cores, node_of_row, PAD_P0)."""
    loops = np.arange(N, dtype=np.int64)
    src = np.concatenate([edge_index[0].astype(np.int64), loops])
    dst = np.concatenate([edge_index[1].astype(np.int64), loops])
    deg = np.bincount(dst, minlength=NPAD)  # pad nodes have degree 0
    order = np.argsort(-deg, kind="stable")

    BPC = NPAD // (128 * NCORES)
    NPC = NPAD // NCORES
    node_of_row = np.empty(NPAD, dtype=np.int64)
    for b in range(BPC - 1):
        chunk = order[b * 1024:(b + 1) * 1024]
        q = np.arange(1024)
        rows = (q // 128) * NPC + b * 128 + (q % 128)
        node_of_row[rows] = chunk
    # last chunk: reals first on every core, pads fill the tail partitions
    last = order[(BPC - 1) * 1024:]
    n_real = int((deg[last] > 0).sum())
    assert n_real % NCORES == 0
    reals, pads = last[:n_real], last[n_real:]
    rpc = n_real // NCORES
    ppc = (1024 - n_real) // NCORES
    b = BPC - 1
    for c in range(NCORES):
        base = c * NPC + b * 128
        node_of_row[base:base + rpc] = reals[c * rpc:(c + 1) * rpc]
        node_of_row[base + rpc:base + 128] = pads[c * ppc:(c + 1) * ppc]
    row_of_node = np.empty(NPAD, dtype=np.int64)
    row_of_node[node_of_row] = np.arange(NPAD)
    PAD_P0 = rpc
    PADROW = NPAD  # dedicated poison row appended past the node table

    Dpad = []
    for b in range(BPC):
        mx = int(deg[order[b * 1024]])
        Dpad.append(max(1, mx))

    sidx = np.argsort(dst, kind="stable")
    src_s, dst_s = src[sidx], dst[sidx]
    starts = np.searchsorted(dst_s, np.arange(NPAD))
    ends = np.searchsorted(dst_s, np.arange(NPAD) + 1)

    idx_cores = []
    for c in range(NCORES):
        cols = []
        for b in range(BPC):
            D = Dpad[b]
            flat = np.full(D * 128, PADROW, dtype=np.int64)
            for p in range(128):
                v = node_of_row[c * NPC + b * 128 + p]
                s0, s1 = starts[v], ends[v]
                k = s1 - s0
                if k:
                    flat[np.arange(k) * 128 + p] = row_of_node[src_s[s0:s1]]
            cols.append(_wrap16(flat))
        idx_cores.append(np.ascontiguousarray(np.concatenate(cols, axis=1)))
    return tuple(Dpad), idx_cores, node_of_row, PAD_P0


def prep_values(x, Ws, a_srcs, a_dsts, NPAD, node_of_row):
    N, F = x.shape
    xp = np.zeros((NPAD, F), dtype=np.float32)
    xp[:N] = x
    xperm = xp[node_of_row]  # row r holds node node_of_row[r]
    xT = np.ascontiguousarray(xperm.T).astype(BF16)  # [F, NPAD]

    W_augs = []
    for W, a_s, a_d in zip(Ws, a_srcs, a_dsts):
        H, Fin, C = W.shape
        RW = _row_width(H, C)
        FW = H * (C + 1)
        wsrc = np.einsum("hfc,hc->fh", W, a_s)
        wdst = np.einsum("hfc,hc->fh", W, a_d)
        Wa = np.zeros((Fin, RW), dtype=np.float32)
        for h in range(H):
            # col h*(C+1)+C stays 0: the ones column, memset on device
            Wa[:, h * (C + 1):h * (C + 1) + C] = W[h].reshape(Fin, C)
        Wa[:, FW:FW + H] = wsrc
        Wa[:, FW + H:FW + 2 * H] = wdst
        W_augs.append(Wa.astype(BF16))
    return xT, W_augs


# ----------------------------------------------------------------------------
# Device program
# ----------------------------------------------------------------------------

def build_nc(cfg, repeat=1):
    import concourse.bacc as bacc
    import concourse.mybir as mybir
    import concourse.tile as tile
    from contextlib import ExitStack

    f32 = mybir.dt.float32
    bf16 = mybir.dt.bfloat16
    i16 = mybir.dt.int16
    ALU = mybir.AluOpType
    ACT = mybir.ActivationFunctionType
    AX = mybir.AxisListType

    N = cfg["N"]
    NPAD = cfg["NPAD"]
    F_IN = cfg["F_IN"]
    C = cfg["C"]
    Dpad = cfg["Dpad"]
    HS = cfg["HS"]
    PAD_P0 = cfg["PAD_P0"]
    BPC = NPAD // (128 * NCORES)
    NPC = NPAD // NCORES
    NL = len(HS)
    RWs = [_row_width(H, C) for H in HS]
    FINs = [F_IN] + [C] * (NL - 1)
    DSUM = sum(Dpad)
    doff = [0]
    for d in Dpad:
        doff.append(doff[-1] + d)
    Dmax = max(Dpad)
    G1W = Dmax * max(RWs)

    nc = bacc.Bacc("TRN2", target_bir_lowering=False, debug=False,
                   num_devices=NCORES)

    xT_d = nc.dram_tensor("xT", [F_IN, NPC], bf16, kind="ExternalInput")
    idx_d = nc.dram_tensor("idxs", [128, DSUM * 8], i16, kind="ExternalInput")
    W_d = [nc.dram_tensor(f"w{i+1}", [FINs[i], RWs[i]], bf16,
                          kind="ExternalInput") for i in range(NL)]
    bb_d = [nc.dram_tensor(f"bb{i+1}", [128, C], f32, kind="ExternalInput")
            for i in range(NL - 1)]
    out_d = nc.dram_tensor("out", [1, C], f32, kind="ExternalOutput")

    with tile.TileContext(nc, num_cores=NCORES) as tc, ExitStack() as ctx:
        dram = ctx.enter_context(tc.tile_pool(name="dram", bufs=1, space="DRAM"))
        cpool = ctx.enter_context(tc.tile_pool(name="consts", bufs=1))
        hpool = ctx.enter_context(tc.tile_pool(name="hs", bufs=1))
        wpool = ctx.enter_context(tc.tile_pool(name="work", bufs=1))
        psum = ctx.enter_context(tc.tile_pool(name="ps", bufs=2, space="PSUM"))

        hl = [dram.tile([NPC, RWs[i]], bf16, tag=f"hl{i}", name=f"hl{i}")
              for i in range(NL)]
        # one extra row past the node table: the poison row pad slots point at
        hf = [dram.tile([NPAD + 1, RWs[i]], bf16, tag=f"hf{i}", name=f"hf{i}")
              for i in range(NL)]
        x2d = [dram.tile([NPC, C], bf16, tag=f"x2d{i}", name=f"x2d{i}")
               for i in range(NL - 1)]

        xT_sb = cpool.tile([F_IN, NPC], bf16, tag="xT")
        nc.sync.dma_start(xT_sb[:], xT_d[:, :])
        idx_sb = cpool.tile([128, DSUM * 8], i16, tag="idx")
        nc.sync.dma_start(idx_sb[:], idx_d[:, :])
        W_sb = []
        for i in range(NL):
            w = cpool.tile([FINs[i], RWs[i]], bf16, tag=f"w{i}", name=f"w{i}")
            nc.sync.dma_start(w[:], W_d[i][:, :])
            W_sb.append(w)
        bb_sb = []
        for i in range(NL - 1):
            t = cpool.tile([128, C], f32, tag=f"bb{i}", name=f"bb{i}")
            nc.sync.dma_start(t[:], bb_d[i][:, :])
            bb_sb.append(t)
        ones_sb = cpool.tile([128, 1], f32, tag="ones")
        nc.vector.memset(ones_sb[:], 1.0)
        pois_sb = cpool.tile([1, max(RWs)], bf16, tag="pois")
        nc.vector.memset(pois_sb[:], -1e9)
        x2T = [cpool.tile([C, NPC], bf16, tag=f"x2T{i}", name=f"x2T{i}")
               for i in range(NL - 1)]
        g1f = cpool.tile([128, G1W], bf16, tag="g1f")
        msgf = cpool.tile([128, max(HS) * (C + 1) * Dmax], bf16, tag="msgf")

        pfin = psum.tile([1, C], f32, tag="pfin", bufs=1)
        dreg = {d: nc.gpsimd.to_reg(d * 128) for d in sorted(set(Dpad))}

        for _rep in range(repeat):
         for L in range(NL):
            H = HS[L]
            RW = RWs[L]
            FW = H * (C + 1)

            # ---- phase A: h_aug for own nodes ----
            if L > 0:
                # transposed activations via DMA XBAR transpose
                nc.sync.dma_start(x2T[L - 1][:, :], x2d[L - 1][:, :],
                                  transpose=True)
            lsrc = xT_sb if L == 0 else x2T[L - 1]
            for g0 in range(0, BPC, 4):
                hs4 = hpool.tile([128, 4, RW], bf16, tag="hs4")
                if RW > 512:
                    pA4 = psum.tile([128, 4, 512], f32, tag="pA4", bufs=1)
                    pB4 = psum.tile([128, 4, RW - 512], f32, tag="pB4", bufs=1)
                else:
                    pA4 = psum.tile([128, 4, RW], f32, tag="pA4", bufs=1)
                for j in range(4):
                    lhsT = lsrc[:, (g0 + j) * 128:(g0 + j + 1) * 128]
                    if RW > 512:
                        nc.tensor.matmul(pA4[:, j, :], lhsT, W_sb[L][:, 0:512],
                                         start=True, stop=True)
                        nc.tensor.matmul(pB4[:, j, :], lhsT, W_sb[L][:, 512:RW],
                                         start=True, stop=True)
                    else:
                        nc.tensor.matmul(pA4[:, j, :], lhsT, W_sb[L][:, 0:RW],
                                         start=True, stop=True)
                if RW > 512:
                    nc.scalar.copy(hs4[:, :, 0:512], pA4[:])
                    nc.vector.tensor_copy(hs4[:, :, 512:RW], pB4[:])
                else:
                    nc.scalar.copy(hs4[:], pA4[:])
                ones_ap = hs4[:, :, 0:FW].rearrange(
                    "p j (h c) -> p j h c", c=C + 1)[:, :, :, C]
                nc.vector.memset(ones_ap, 1.0)
                dst_ap = hl[L][g0 * 128:(g0 + 4) * 128, :].rearrange(
                    "(j p) w -> p j w", j=4)
                nc.sync.dma_start(dst_ap, hs4[:])

            # ---- phase B: allgather + poison row ----
            nc.gpsimd.collective_compute(
                "AllGather", mybir.AluOpType.bypass,
                replica_groups=[list(range(NCORES))],
                ins=[hl[L][:, :].opt()],
                outs=[hf[L][0:NPAD, :].opt()],
            )
            nc.sync.dma_start(hf[L][NPAD:NPAD + 1, :], pois_sb[:, 0:RW])
            # own nodes' s_dst, [128, BPC, H]
            sdl = wpool.tile([128, BPC, H], bf16, tag="sdl")
            hl_ap = hl[L][:, :].rearrange("(b p) w -> p b w", p=128)
            nc.sync.dma_start(sdl[:], hl_ap[:, :, FW + H:FW + 2 * H])

            # ---- phase C: per dst block (numer/denom), tail batched ----
            nma = wpool.tile([128, BPC, H, C + 1], bf16, tag="nma")
            for b in range(BPC):
                D = Dpad[b]
                g1 = g1f[:].rearrange("p (d w) -> p d w", w=RW)[:, 0:D, :]
                nc.gpsimd.dma_gather(
                    g1[:, :, :], hf[L][:, :],
                    idx_sb[:, doff[b] * 8:(doff[b] + D) * 8],
                    D * 128, dreg[D], RW, single_packet=False)
                p = wpool.tile([128, D, H], f32, tag="p")
                sc = wpool.tile([128, D, H], f32, tag="sc")
                nc.vector.tensor_tensor(
                    sc[:], g1[:, :, FW:FW + H],
                    sdl[:, b:b + 1, :].broadcast_to([128, D, H]), ALU.add)
                lr = wpool.tile([128, D, H], f32, tag="lr")
                nc.vector.scalar_tensor_tensor(lr[:], sc[:], 0.2, sc[:],
                                               op0=ALU.mult, op1=ALU.max)
                nc.scalar.activation(p[:], lr[:], ACT.Exp)

                msg = msgf[:, 0:H * (C + 1) * D].rearrange(
                    "p (h c d) -> p h c d", h=H, c=C + 1)
                g1v = g1[:, :, 0:FW].rearrange("p d (h c) -> p h c d", h=H)
                pv = p[:].rearrange("p d h -> p h d").unsqueeze(2).broadcast_to(
                    [128, H, C + 1, D])
                nc.vector.tensor_tensor(msg, g1v, pv, ALU.mult)
                with nc.allow_low_precision("bf16 numer accumulate"):
                    nc.vector.tensor_reduce(nma[:, b, :, :], msg, axis=AX.X,
                                            op=ALU.add)

            # ---- batched tail over all BPC blocks ----
            dna = wpool.tile([128, BPC, H], f32, tag="dna")
            nc.vector.tensor_scalar(dna[:], nma[:, :, :, C], float(H),
                                    1e-16 * H, op0=ALU.mult, op1=ALU.add)
            rca = wpool.tile([128, BPC, H], f32, tag="rca")
            nc.vector.reciprocal(rca[:], dna[:])
            g1_f32 = g1f[:].bitcast(f32)
            if L < NL - 1:
                nm2a = msgf[:, 0:BPC * H * C].rearrange(
                    "p (b h c) -> p b h c", b=BPC, h=H)
                nc.vector.tensor_tensor(
                    nm2a, nma[:, :, :, 0:C],
                    rca[:].unsqueeze(3).broadcast_to([128, BPC, H, C]),
                    ALU.mult)
                xoa = g1_f32[:, 0:BPC * C].rearrange("p (b c) -> p b c", b=BPC)
                nc.vector.tensor_reduce(xoa,
                                        nm2a.rearrange("p b h c -> p b c h"),
                                        axis=AX.X, op=ALU.add)
                xba = g1_f32[:, BPC * C:2 * BPC * C].rearrange(
                    "p (b c) -> p b c", b=BPC)
                nc.vector.tensor_tensor(
                    xba, xoa,
                    bb_sb[L][:].unsqueeze(1).broadcast_to([128, BPC, C]),
                    ALU.add)
                x2ba = g1f[:, 4 * BPC * C:5 * BPC * C].rearrange(
                    "p (b c) -> p b c", b=BPC)
                nc.vector.tensor_scalar(x2ba, xba, 0.0, None, op0=ALU.max)
                nc.sync.dma_start(
                    x2d[L][:, :].rearrange("(b p) c -> p b c", p=128), x2ba)
            else:
                o3a = g1_f32[:, 0:BPC * C].rearrange("p (b c) -> p b c", b=BPC)
                nc.vector.tensor_tensor(
                    o3a, nma[:, :, 0, 0:C],
                    rca[:, :, 0:1].broadcast_to([128, BPC, C]), ALU.mult)
                o3s = wpool.tile([128, C], f32, tag="o3s")
                nc.vector.tensor_reduce(o3s[:],
                                        o3a.rearrange("p b c -> p c b"),
                                        axis=AX.X, op=ALU.add)
                nc.tensor.matmul(pfin[:], ones_sb[:], o3s[:],
                                 start=True, stop=True)

        fs = wpool.tile([1, C], f32, tag="fs")
        nc.scalar.copy(fs[:], pfin[:])
        nc.sync.dma_start(out_d[:, :], fs[:])

    nc.compile()
    return nc


# ----------------------------------------------------------------------------
# Entry points
# ----------------------------------------------------------------------------

_PREP_CACHE = {}


def make_cfg_and_maps(inputs):
    x = np.asarray(inputs["x"])
    edge_index = np.asarray(inputs["edge_index"])
    N, F_IN = x.shape
    NPAD = ((N + 1023) // 1024) * 1024
    Ws = [np.asarray(inputs[f"W{i}"]) for i in (1, 2, 3)]
    a_srcs = [np.asarray(inputs[f"as{i}"]) for i in (1, 2, 3)]
    a_dsts = [np.asarray(inputs[f"ad{i}"]) for i in (1, 2, 3)]
    bs = [np.asarray(inputs[f"b{i}"]) for i in (1, 2, 3)]
    HS = tuple(W.shape[0] for W in Ws)
    C = Ws[0].shape[2]

    pkey = (id(inputs["edge_index"]), edge_index.shape, N)
    if pkey not in _PREP_CACHE:
        _PREP_CACHE.clear()
        _PREP_CACHE[pkey] = prep_static(edge_index, N, NPAD)
    Dpad, idx_cores, node_of_row, PAD_P0 = _PREP_CACHE[pkey]
    xT, W_augs = prep_values(x, Ws, a_srcs, a_dsts, NPAD, node_of_row)

    cfg = dict(N=N, NPAD=NPAD, F_IN=F_IN, C=C, Dpad=Dpad, HS=HS,
               PAD_P0=PAD_P0)
    b3 = bs[2].astype(np.float32).reshape(1, C)
    NPC = NPAD // NCORES
    in_maps = []
    for c in range(NCORES):
        m = {
            "xT": np.ascontiguousarray(xT[:, c * NPC:(c + 1) * NPC]),
            "idxs": idx_cores[c],
        }
        for i in range(3):
            m[f"w{i+1}"] = W_augs[i]
        for i in range(2):
            m[f"bb{i+1}"] = np.ascontiguousarray(
                np.broadcast_to(bs[i].astype(np.float32)[None, :], (128, C)))
        in_maps.append(m)
    return cfg, in_maps, b3


_NC_CACHE = {}


def _get_nc(cfg, repeat=1):
    key = (repeat,) + tuple(sorted((k, v) for k, v in cfg.items()))
    if key not in _NC_CACHE:
        _NC_CACHE[key] = build_nc(cfg, repeat=repeat)
    return _NC_CACHE[key]


def run(inputs, trace=False, repeat=1, **kw):
    from concourse.bass_utils import run_bass_kernel_spmd
    cfg, in_maps, b3 = make_cfg_and_maps(inputs)
    nc = _get_nc(cfg, repeat=repeat)
    res = run_bass_kernel_spmd(nc, in_maps, core_ids=list(range(NCORES)),
                               trace=trace, **kw)
    out = np.zeros((1, cfg["C"]), dtype=np.float32)
    for r in res.results:
        out += r["out"]
    out = out * (1.0 / cfg["N"]) + b3
    return out, res


def kernel(**inputs) -> np.ndarray:
    out, _ = run(inputs)
    return out
